# revision 26
# baseline (speedup 1.0000x reference)
"""Trainium2 Bass kernel for nn_CustomDecoderLayer (FAVOR+ decoder layer).

Sharding: 8 cores = 4 batches x 2 sequence halves (S'=1024 tokens/core),
full D/heads/F per core. The only collectives are one bf16 AllReduce-add
of the FAVOR+ kv summary (65 x H*258, ~0.5 MB) per attention block,
within each core pair; column 257 carries exp(8*(local_stab-8)) so the
same AllReduce transports the k-stabilizer (recovered via x^(-1/8)).
Both ARs are launched early and covered by independent compute (the
cross-attn k/v path does not depend on self-attn output; its projection
units run as fillers inside the self-attn phi(k) pipeline). The
residual stays in SBUF for the whole kernel; LayerNorm gamma/beta are
folded into the downstream projection weights host-side.

All inputs ride in ONE host-packed bf16 blob per core (f32 slices are
bitcast views) -- per-exec dispatch overhead scales with the input
buffer count (~20us/buffer), so 30 tensors -> 1 saves ~0.6 ms/exec.
"""
import sys
sys.path.insert(0, "/opt/trn_rl_repo")
from contextlib import ExitStack

import numpy as np
import ml_dtypes

import concourse.bass as bass
import concourse.mybir as mybir
import concourse.tile as tile
from concourse import bacc, bass_isa

f32 = mybir.dt.float32
f32r = mybir.dt.float32r
bf16 = mybir.dt.bfloat16
AF = mybir.ActivationFunctionType
AX = mybir.AxisListType
ALU = mybir.AluOpType

D, H, DH, M = 1024, 16, 64, 256
S, B, F = 2048, 4, 4096
SL = 1024                     # tokens per core (one seq half)
KD = D // 128                 # 8
RT = SL // 128                # 8
NCH = SL // 512               # 2
NP = H // 2                   # 8 head pairs
C2 = 0.5 * (DH ** -0.5)       # 0.0625, exact in bf16
EPS16 = 1.0e-6 * 16.0
RG = [[0, 1], [2, 3], [4, 5], [6, 7]]

_CACHE = {}

# ---------------------------------------------------------------- blob
# One bf16 dram tensor per core; (name, bf16-cols, note). f32 payloads
# occupy 2 cols per value and are bitcast device-side.

_BLOB_SPEC = [
    ("xT", 2 * KD * SL),        # f32 (128, KD, SL)
    ("sa_wk", KD * D),          # bf16 (128, KD, D)
    ("sa_bk", 2 * KD),          # f32 (128, KD)
    ("sa_wf", 512),             # bf16 (128, 512)
    ("sa_wv", KD * D),
    ("sa_bv", D),               # bf16 (1, D) in partition 0
    ("memT", KD * SL),          # bf16 (128, KD, SL)
    ("ca_wk", KD * D),
    ("ca_bk", 2 * KD),
    ("ca_wf", 512),
    ("ca_wv", KD * D),
    ("ca_bv", D),
    ("sa_wq", KD * D),
    ("sa_bq", 2 * KD),
    ("c_ident", 128),           # bf16 (128, 128)
    ("sa_wo", KD * D),
    ("sa_bo", 2 * KD),
    ("ca_wq", KD * D),
    ("ca_bq", 2 * KD),
    ("ca_wo", KD * D),
    ("ca_bo", 2 * KD),
    ("w1", 4 * KD * 1024),      # bf16 (128, 4, KD, 1024)
    ("b1", 2 * 32),             # f32 (128, 32)
    ("w2", 4 * KD * 1024),
    ("b2", 2 * KD),             # f32 (128, KD)
]
_BLOB_OFF = {}
_off = 0
for _nm, _c in _BLOB_SPEC:
    _BLOB_OFF[_nm] = _off
    _off += _c
BLOB_COLS = _off


def _bslice(blob, name):
    off = _BLOB_OFF[name]
    cols = dict(_BLOB_SPEC)[name]
    return blob[:, off:off + cols]


def _ln(nc, tc, ctx, x_t, out_t, c_invd, c_eps):
    """LayerNorm F-layout: x_t (128, KD, SL) f32r -> out_t bf16.

    Stats for both chunks first, then normalize kd-outer so consumers
    that read t2 per-kd (full SL) unblock as early as possible. The
    affine gamma/beta are folded into downstream weights host-side, so
    this emits plain (x - mu) * rstd."""
    ps = ctx.enter_context(tc.tile_pool(name="lnps", bufs=2, space="PSUM"))
    sb = ctx.enter_context(tc.tile_pool(name="lnsb", bufs=3))
    mus, rstds = [], []
    for ch in range(NCH):
        cs = bass.ts(ch, 512)
        mv = ps.tile([128, 2, 512], f32, tag="ln_ps", name="ln_ps")
        for kd in range(KD):
            x2 = sb.tile([128, 512], f32r, tag="ln_x2", name="ln_x2")
            nc.scalar.activation(x2[:], x_t[:, kd, cs].bitcast(f32),
                                 AF.Square)
            nc.tensor.matmul(mv[:, 0, :], c_invd[:], x_t[:, kd, cs],
                             start=(kd == 0), stop=(kd == KD - 1),
                             skip_group_check=True)
            nc.tensor.matmul(mv[:, 1, :], c_invd[:], x2[:],
                             start=(kd == 0), stop=(kd == KD - 1),
                             skip_group_check=True)
        mu = sb.tile([128, 512], f32, tag="ln_mu", name="ln_mu", bufs=2)
        nc.any.tensor_copy(mu[:], mv[:, 0, :])
        mu2 = sb.tile([128, 512], f32, tag="ln_mu2", name="ln_mu2")
        nc.vector.tensor_mul(mu2[:], mu[:], mu[:])
        var = sb.tile([128, 512], f32, tag="ln_var", name="ln_var")
        nc.vector.tensor_sub(var[:], mv[:, 1, :], mu2[:])
        sd = sb.tile([128, 512], f32, tag="ln_sd", name="ln_sd")
        nc.scalar.activation(sd[:], var[:], AF.Sqrt, bias=c_eps[:])
        rstd = sb.tile([128, 512], f32, tag="ln_rstd", name="ln_rstd")
        nc.vector.reciprocal(rstd[:], sd[:])
        mus.append(mu)
        rstds.append(rstd)
    for kd in range(KD):
        for ch in range(NCH):
            cs = bass.ts(ch, 512)
            # split normalize units across DVE and Pool (~10:6)
            eng = nc.vector if (kd * NCH + ch) % 8 < 5 else nc.gpsimd
            xm = sb.tile([128, 512], f32, tag="ln_xm", name="ln_xm")
            eng.tensor_sub(xm[:], x_t[:, kd, cs], mus[ch][:])
            eng.tensor_mul(out_t[:, kd, cs], xm[:], rstds[ch][:])


def _load_w(nc, wp, w_ap):
    """One-DMA load of a host-packed (128, KD, 1024) bf16 weight."""
    w_sb = wp.tile([128, KD, D], bf16, tag="w", name="w_sb")
    nc.sync.dma_start(out=w_sb[:], in_=w_ap)
    return w_sb


def _proj_F_unit(nc, ps, src_t, w_sb, b_t, out_t, m, dve_evac=False):
    """One m-tile of an F-layout projection."""
    o_ps = ps.tile([128, SL], f32, tag="proj_ps", name="proj_ps")
    for kd in range(KD):
        for ch in range(NCH):
            cs = bass.ts(ch, 512)
            nc.tensor.matmul(o_ps[:, cs], w_sb[:, kd, m * 128:(m + 1) * 128],
                             src_t[:, kd, cs],
                             start=(kd == 0), stop=(kd == KD - 1),
                             skip_group_check=True)
    nc.any.tensor_scalar_add(out_t[:, m, :], o_ps[:], b_t[:, m:m + 1])


def _proj_F(nc, tc, ctx, src_t, w_sb, b_t, out_t):
    """F-layout projection: out[dout, tok]. src (128, KD, SL) bf16,
    w_sb (128, KD, D) [din, dout]. Full-SL moving operand (bf16)."""
    ps = ctx.enter_context(tc.tile_pool(name="pfps", bufs=2, space="PSUM"))
    for m in range(KD):
        _proj_F_unit(nc, ps, src_t, w_sb, b_t, out_t, m)


def _proj_R_unit(nc, ps, src_t, w_sb, bv_rep, v_aug, rt):
    """One token-tile of the R-layout V projection."""
    rs = bass.ts(rt, 128)
    v_ps = ps.tile([128, SL], f32, tag="v_ps", name="v_ps")
    for kd in range(KD):
        for ch in range(NCH):
            cs = bass.ts(ch, 512)
            nc.tensor.matmul(v_ps[:, cs], src_t[:, kd, rs],
                             w_sb[:, kd, cs],
                             start=(kd == 0), stop=(kd == KD - 1),
                             skip_group_check=True)
    nc.any.tensor_add(v_aug[:, rt, :, 0:64], v_ps[:], bv_rep[:])


def _proj_R_vaug(nc, tc, ctx, src_t, w_sb, bv_rep, v_aug):
    """R-layout V projection into v_aug (128, RT, H, 65) bf16."""
    ps = ctx.enter_context(tc.tile_pool(name="pvps", bufs=2, space="PSUM"))
    for rt in range(RT):
        _proj_R_unit(nc, ps, src_t, w_sb, bv_rep, v_aug, rt)


def _phi_k_kv(nc, tc, ctx, kT, v_aug, wf_cat, c_negblk, c_b64, kv_sb,
              fillers=(), half_cb=None, ebufs=2, kvbufs=1, pjbufs=2):
    """phi(k) + local kv partials, all heads. E carries no stabilizer;
    row 65 of kv_sb gets exp(8*(local_stab - 8)) so the single
    AllReduce-add also transports the stab: the consume side recovers
    ~exp(-max_stab) as (sum)^(-1/8) * e^-8 (error <= log2/8 in the
    exponent, which only perturbs the eps weighting by <1%).

    `fillers` are independent PE-dense work units (closures) interleaved
    between pipeline stages; each pair's kv matmuls are emitted one pair
    late so the in-order PE queue never waits on the Act-engine exps."""
    ps_pj = ctx.enter_context(tc.tile_pool(name="pkpj", bufs=pjbufs,
                                           space="PSUM"))
    ps_ns = ctx.enter_context(tc.tile_pool(name="pkns", bufs=1, space="PSUM"))
    ps_kv = ctx.enter_context(tc.tile_pool(name="pkkv", bufs=kvbufs,
                                           space="PSUM"))
    sb = ctx.enter_context(tc.tile_pool(name="pksb", bufs=3))
    ep = ctx.enter_context(tc.tile_pool(name="pkep", bufs=ebufs))
    nc.vector.memset(kv_sb[:, :, 257:258], 0.0)
    fill_iter = iter(fillers)

    def emit_fill(n):
        for _ in range(n):
            f = next(fill_iter, None)
            if f is not None:
                f()

    def stage_kv(g, E_t):
        for h in range(2):
            kv_ps = ps_kv.tile([65, 257], f32, tag="kv_ps", name="kv_ps")
            for rt in range(RT):
                nc.tensor.matmul(kv_ps[:], v_aug[:, rt, 2 * g + h, :],
                                 E_t[:, rt, h, :], start=(rt == 0),
                                 stop=(rt == RT - 1), skip_group_check=True)
            nc.any.tensor_copy(kv_sb[0:65, 2 * g + h, 0:257], kv_ps[:])

    prev = None
    for g in range(NP):
        E_t = ep.tile([128, RT, 2, 257], bf16, tag="E_t", name="E_t")
        nc.vector.memset(E_t[:, :, :, 256:257], EPS16)
        nsq_all = sb.tile([128, RT, 2], f32, tag="nsq_all", name="nsq_all")
        for rt in range(RT):
            rs = bass.ts(rt, 128)
            k2 = sb.tile([128, 128], bf16, tag="k2", name="k2")
            nc.vector.tensor_mul(k2[:], kT[:, g, rs], kT[:, g, rs])
            pj = ps_pj.tile([128, 2, 256], f32, tag="pj", name="pj_k")
            nc.tensor.matmul(pj[:, :, :], kT[:, g, rs], wf_cat[:])
            nsq = ps_ns.tile([128, 2], f32, tag="nsq", name="nsq_k")
            nc.tensor.matmul(nsq[:], k2[:], c_negblk[:])
            nc.any.tensor_copy(nsq_all[:, rt, :], nsq[:])
            for h in range(2):
                nc.scalar.activation(E_t[:, rt, h, 0:256], pj[:, h, :],
                                     AF.Exp, bias=nsq_all[:, rt, h:h + 1])
        # stabilizer recovered post-exp: max_m E = e^(stab_tok - sq), so
        # stab_tok = ln(max_m E) + sq; one wide bf16 reduce replaces 8
        # PSUM f32 reduce_max ops.
        maxE = sb.tile([128, RT, 2], f32, tag="maxE", name="maxE")
        nc.vector.reduce_max(maxE[:], E_t[:, :, :, 0:256], axis=AX.X)
        lnE = sb.tile([128, RT, 2], f32, tag="lnE", name="lnE")
        nc.scalar.activation(lnE[:], maxE[:], AF.Ln)
        stab_tok = sb.tile([128, RT, 2], f32, tag="stab_tok",
                           name="stab_tok")
        nc.vector.tensor_sub(stab_tok[:], lnE[:], nsq_all[:])
        stab_run = sb.tile([128, 2], f32, tag="stab_run", name="stab_run")
        nc.vector.reduce_max(stab_run[:],
                             stab_tok[:].rearrange("p r h -> p h r"),
                             axis=AX.X)
        stab_rep = sb.tile([128, 2], f32, tag="stab_rep", name="stab_rep")
        nc.gpsimd.partition_all_reduce(stab_rep[:], stab_run[:], channels=128,
                                       reduce_op=bass_isa.ReduceOp.max)
        nc.scalar.activation(kv_sb[0:1, 2 * g:2 * g + 2, 257:258],
                             stab_rep[0:1, :], AF.Exp, scale=8.0,
                             bias=c_b64[0:1, :])
        emit_fill(2)
        if prev is not None:
            stage_kv(*prev)
            if prev[0] == 3 and half_cb is not None:
                half_cb(0)
        prev = (g, E_t)
    stage_kv(*prev)
    if half_cb is not None:
        half_cb(1)
    emit_fill(len(fillers))


def _kv_consume(nc, tc, ctx, kv_out, ident_bf, kvT, kvcolT, h0=0, nh=H):
    """Load AR result; recover s = ~exp(-stab_max) from the summed
    exp(8*(stab-8)) row via three chained sqrts + fast reciprocal;
    scale kv by s and fold in the (unscaled) eps column; transpose
    -> kvT. Also emits kvcolT[1, h, 65] = sum_m kva (the rank-1
    stationary for the exact q-side eps correction)."""
    sb = ctx.enter_context(tc.tile_pool(name="kcsb", bufs=2))
    kvp = ctx.enter_context(tc.tile_pool(name="kckv", bufs=1))
    ps_tp = ctx.enter_context(tc.tile_pool(name="kctp", bufs=2, space="PSUM"))
    kv2 = kvp.tile([65, nh, 258], bf16, name="kv2")
    nc.sync.dma_start(out=kv2[:], in_=kv_out[:])
    s_row = sb.tile([1, nh], f32, tag="s_row", name="s_row")
    nc.any.tensor_copy(s_row[:], kv2[0:1, :, 257:258])
    r1 = sb.tile([1, nh], f32, tag="r1", name="r1")
    nc.scalar.activation(r1[:], s_row[:], AF.Sqrt)
    r2 = sb.tile([1, nh], f32, tag="r2", name="r2")
    nc.scalar.activation(r2[:], r1[:], AF.Sqrt)
    r3 = sb.tile([1, nh], f32, tag="r3", name="r3")
    nc.scalar.activation(r3[:], r2[:], AF.Sqrt)
    r4 = sb.tile([1, nh], f32, tag="r4", name="r4")
    nc.vector.reciprocal(r4[:], r3[:])
    s_t = sb.tile([1, nh], f32, tag="s_t", name="s_t")
    nc.vector.tensor_scalar_mul(s_t[:], r4[:], float(np.exp(-8.0)))
    s_bc = sb.tile([128, nh], f32, tag="s_bc", name="s_bc")
    nc.gpsimd.partition_broadcast(s_bc[:], s_t[:], channels=128)
    for h in range(nh):
        csum = sb.tile([65, 1], f32, tag="csum", name="csum")
        nc.any.tensor_copy(csum[:], kv2[0:65, h, 256:257])
        kva = sb.tile([65, 256], bf16, tag="kva", name="kva")
        nc.vector.tensor_scalar(kva[:], kv2[0:65, h, 0:256],
                                s_bc[0:65, h:h + 1], csum[:],
                                ALU.mult, ALU.add)
        kvcol = sb.tile([65, 1], f32, tag="kvcol", name="kvcol")
        nc.vector.reduce_sum(kvcol[:], kva[:], axis=AX.X)
        kvcolb = sb.tile([65, 1], bf16, tag="kvcolb", name="kvcolb")
        nc.any.tensor_copy(kvcolb[:], kvcol[:])
        tpc = ps_tp.tile([1, 65], bf16, tag="tp_kv", name="tp_kvc")
        nc.tensor.transpose(tpc[:], kvcolb[:], ident_bf[0:65, 0:65])
        nc.any.tensor_copy(kvcolT[0:1, h0 + h, :], tpc[:])
        for mt in range(2):
            tp = ps_tp.tile([128, 65], bf16, tag="tp_kv", name="tp_kv")
            nc.tensor.transpose(tp[:], kva[0:65, mt * 128:(mt + 1) * 128],
                                ident_bf[0:65, 0:65])
            nc.any.tensor_copy(kvT[:, h0 + h, mt, :], tp[:])


def _phi_q_out(nc, tc, ctx, qT, kvT, kvcolT, wf_cat, c_negblk, ident_bf,
               c_lneps, attn_t, pairs=range(NP)):
    """phi(q), exact reference semantics, stabilizer-free.

    A per-token scale on pq cancels exactly in out/z, so pq is used
    UNSCALED: pq_u = e^(proj - sq) (bf16-safe, <= e^5.3 here). The only
    place the reference stabilizer matters is the relative weight of
    its +eps term, which equals a rank-1 correction
    eps16 * e^(stab_tok) * colsum_m(kva) -- added exactly via a K=1
    matmul accumulated into the same PSUM group (e^(stab_tok) =
    rowmax(pq_u) * e^(sq), both cheap post-exp byproducts). This
    removes all per-rt PSUM reductions and the scale barrier from the
    q path."""
    ps_pj = ctx.enter_context(tc.tile_pool(name="pqpj", bufs=2, space="PSUM"))
    ps_ns = ctx.enter_context(tc.tile_pool(name="pqns", bufs=1, space="PSUM"))
    ps_tp = ctx.enter_context(tc.tile_pool(name="pqtp", bufs=3, space="PSUM"))
    ps_o = ctx.enter_context(tc.tile_pool(name="pqo", bufs=2, space="PSUM"))
    sb = ctx.enter_context(tc.tile_pool(name="pqsb", bufs=3))
    pqrp = ctx.enter_context(tc.tile_pool(name="pqrp", bufs=2))
    pqp = ctx.enter_context(tc.tile_pool(name="pqpq", bufs=2))
    for g in pairs:
        pqR = pqrp.tile([128, RT, 2, 256], bf16, tag="pqR", name="pqR")
        nsq_ps = ps_ns.tile([128, RT, 2], f32, tag="nsq", name="nsq_q")
        rmax = sb.tile([128, RT, 2], f32, tag="rmax_q", name="rmax_q")
        enq = sb.tile([128, RT, 2], f32, tag="enq", name="enq")
        eff = sb.tile([128, RT, 2], bf16, tag="eff", name="eff")
        effT = sb.tile([1, 2, SL], bf16, tag="effT", name="effT", bufs=2)
        pqT = pqp.tile([128, 2, 2, SL], bf16, tag="pqT", name="pqT")
        for rt in range(RT):
            rs = bass.ts(rt, 128)
            q2 = sb.tile([128, 128], bf16, tag="q2", name="q2")
            nc.vector.tensor_mul(q2[:], qT[:, g, rs], qT[:, g, rs])
            pj = ps_pj.tile([128, 2, 256], f32, tag="pj", name="pj_q")
            nc.tensor.matmul(pj[:, :, :], qT[:, g, rs], wf_cat[:])
            nc.tensor.matmul(nsq_ps[:, rt, :], q2[:], c_negblk[:])
            nc.scalar.activation(pqR[:, rt, :, :], pj[:, :, :], AF.Exp)
            tp = ps_tp.tile([128, 2, 2, 128], bf16, tag="tp_pq", name="tp_pq")
            for h in range(2):
                for mt in range(2):
                    nc.tensor.transpose(
                        tp[:, h, mt, :],
                        pqR[:, rt, h, mt * 128:(mt + 1) * 128], ident_bf[:])
            nc.any.tensor_copy(pqT[:, :, :, rs], tp[:])
            if rt % 4 == 3:
                # rank-1 eps factors for this 512-token half:
                # eff = eps16 * e^(stab_tok) = rowmax(pq_u) * eps16*e^(sq)
                hh = slice(rt - 3, rt + 1)
                nc.vector.reduce_max(rmax[:, hh, :], pqR[:, hh, :, :],
                                     axis=AX.X)
                nc.scalar.activation(enq[:, hh, :], nsq_ps[:, hh, :],
                                     AF.Exp, scale=-1.0, bias=c_lneps[:])
                nc.vector.tensor_mul(eff[:, hh, :], rmax[:, hh, :],
                                     enq[:, hh, :])
                for rr in range(rt - 3, rt + 1):
                    tpe = ps_tp.tile([1, 2, 128], bf16, tag="tp_pq",
                                     name="tp_eff")
                    for h in range(2):
                        nc.tensor.transpose(tpe[:, h, :],
                                            eff[:, rr, h:h + 1],
                                            ident_bf[:])
                    nc.any.tensor_copy(effT[:, :, bass.ts(rr, 128)],
                                       tpe[:])
        for h in range(2):
            hp = slice(64 * h, 64 * h + 64)
            for ch in range(NCH):
                cs = bass.ts(ch, 512)
                o_ps = ps_o.tile([65, 512], f32, tag="o_ps", name="o_ps")
                for mt in range(2):
                    nc.tensor.matmul(o_ps[:], kvT[:, 2 * g + h, mt, :],
                                     pqT[:, h, mt, cs], start=(mt == 0),
                                     stop=False, skip_group_check=True)
                nc.tensor.matmul(o_ps[:], kvcolT[0:1, 2 * g + h, :],
                                 effT[0:1, h, cs], start=False, stop=True,
                                 skip_group_check=True)
                zr = sb.tile([1, 512], f32, tag="zr", name="zr")
                nc.vector.reciprocal(zr[:], o_ps[64:65, :])
                zb = sb.tile([64, 512], f32, tag="zb", name="zb", bufs=2)
                nc.gpsimd.partition_broadcast(zb[:], zr[:], channels=64)
                nc.any.tensor_mul(attn_t[hp, g, cs], o_ps[0:64, :],
                                  zb[:])


def _proj_add(nc, tc, ctx, src_t, w_sb, b_t, x_t):
    """Wo-style projection (bf16 src); adds result into x_t (f32r)."""
    ps = ctx.enter_context(tc.tile_pool(name="waps", bufs=2, space="PSUM"))
    for m in range(KD):
        o_ps = ps.tile([128, SL], f32, tag="wa_ps", name="wa_ps")
        for kd in range(KD):
            for ch in range(NCH):
                cs = bass.ts(ch, 512)
                nc.tensor.matmul(o_ps[:, cs],
                                 w_sb[:, kd, m * 128:(m + 1) * 128],
                                 src_t[:, kd, cs],
                                 start=(kd == 0), stop=(kd == KD - 1),
                                 skip_group_check=True)
        nc.vector.scalar_tensor_tensor(
            x_t[:, m, :], o_ps[:], b_t[:, m:m + 1], x_t[:, m, :],
            ALU.add, ALU.add)


def build_nc(no_ar=False):
    nc = bacc.Bacc("TRN2", target_bir_lowering=False, debug=False,
                   num_devices=8)

    blob = nc.dram_tensor("blob", [128, BLOB_COLS], bf16,
                          kind="ExternalInput").ap()

    def wslice(name):
        return _bslice(blob, name).rearrange("p (k d) -> p k d", k=KD)

    def fslice(name, n):
        return _bslice(blob, name).bitcast(f32)

    outT = nc.dram_tensor("outT", [128, KD, SL], f32,
                          kind="ExternalOutput").ap()

    with tile.TileContext(nc) as tc:
        with ExitStack() as top:
            dram = top.enter_context(tc.tile_pool(name="dram", bufs=1,
                                                  space="DRAM"))
            ccs = {}
            for half in "ab":
                ccs["sa_kv_in_" + half] = dram.tile(
                    [65, 8 * 258], bf16, name="sa_kv_in_" + half)
                ccs["sa_kv_out_" + half] = dram.tile(
                    [65, 8 * 258], bf16, name="sa_kv_out_" + half)
            ccs["ca_kv_in"] = dram.tile([65, H * 258], bf16,
                                        name="ca_kv_in")
            ccs["ca_kv_out"] = dram.tile([65, H * 258], bf16,
                                         name="ca_kv_out")

            # persistent activations first so their DMAs lead the queue
            const = top.enter_context(tc.tile_pool(name="const", bufs=1))
            xp = top.enter_context(tc.tile_pool(name="xp", bufs=1))
            x_t = xp.tile([128, KD, SL], f32r, name="x_t")
            xT = _bslice(blob, "xT").bitcast(f32r).rearrange(
                "p (k s) -> p k s", k=KD)
            for kd in range(KD):
                nc.sync.dma_start(out=x_t[:, kd, :], in_=xT[:, kd, :])

            wp = top.enter_context(tc.tile_pool(name="wp", bufs=2))
            # SA front weights ride right behind x so the first
            # projection is never DMA-starved; memory comes after.
            w_k = _load_w(nc, wp, wslice("sa_wk"))
            cb = {}
            for pre in ("sa", "ca"):
                for nm in ("bq", "bk", "bo"):
                    key = pre + "_" + nm
                    t = const.tile([128, KD], f32, name=pre + nm)
                    if key == "sa_bk":
                        nc.sync.dma_start(out=t[:], in_=fslice(key, KD))
                    cb[key] = t
                t = const.tile([1, D], bf16, name=pre + "bv")
                cb[pre + "_bv"] = t
                wfc = const.tile([128, 512], bf16, name=pre + "wfc")
                cb[pre + "_wf"] = wfc
            nc.sync.dma_start(out=cb["sa_wf"][:], in_=_bslice(blob, "sa_wf"))
            w_v = _load_w(nc, wp, wslice("sa_wv"))
            nc.sync.dma_start(out=cb["sa_bv"][:],
                              in_=_bslice(blob, "sa_bv")[0:1, :])
            memstack = ExitStack()
            memp = memstack.enter_context(tc.tile_pool(name="memp", bufs=1,
                                                       side="right"))
            mem_t = memp.tile([128, KD, SL], bf16, name="mem_t")
            nc.sync.dma_start(out=mem_t[:],
                              in_=_bslice(blob, "memT").rearrange(
                                  "p (k s) -> p k s", k=KD))
            for key in ("sa_bq", "sa_bo", "ca_bq", "ca_bk", "ca_bo"):
                nc.sync.dma_start(out=cb[key][:], in_=fslice(key, KD))
            nc.sync.dma_start(out=cb["ca_wf"][:], in_=_bslice(blob, "ca_wf"))
            nc.sync.dma_start(out=cb["ca_bv"][:],
                              in_=_bslice(blob, "ca_bv")[0:1, :])

            # on-device constants (no DMA)
            c_invd = const.tile([128, 128], f32, name="c_invd")
            nc.vector.memset(c_invd[:], 1.0 / D)
            c_invd_r = c_invd[:].bitcast(f32r)
            c_negblk = const.tile([128, 2], bf16, name="c_negblk")
            nc.vector.memset(c_negblk[:], 0.0)
            nc.vector.memset(c_negblk[0:64, 0:1], -C2)
            nc.vector.memset(c_negblk[64:128, 1:2], -C2)
            ident_bf = const.tile([128, 128], bf16, name="ident_bf")
            nc.sync.dma_start(out=ident_bf[:], in_=_bslice(blob, "c_ident"))
            c_eps = const.tile([128, 1], f32, name="c_eps")
            nc.vector.memset(c_eps[:], 1.0e-5)
            c_lneps = const.tile([128, 1], f32, name="c_lneps")
            nc.vector.memset(c_lneps[:], float(np.log(EPS16)))
            c_b64 = const.tile([128, 1], f32, name="c_b64")
            nc.vector.memset(c_b64[:], -64.0)
            for nm, shp in (("b1", [128, 32]), ("b2", [128, KD])):
                t = const.tile(shp, f32, name=nm)
                nc.sync.dma_start(out=t[:], in_=fslice(nm, shp[1]))
                cb[nm] = t

            t2_t = xp.tile([128, KD, SL], bf16, name="t2_t")

            def launch_ar(pre, kv_sb):
                nc.sync.dma_start(out=ccs[pre + "_kv_in"][:], in_=kv_sb[:])
                if no_ar:
                    nc.sync.dma_start(out=ccs[pre + "_kv_out"][:],
                                      in_=ccs[pre + "_kv_in"][:])
                else:
                    nc.gpsimd.collective_compute(
                        "AllReduce", ALU.add, replica_groups=RG,
                        ins=[ccs[pre + "_kv_in"].opt()],
                        outs=[ccs[pre + "_kv_out"].opt()])

            # ---------- LN1 + SA front
            with ExitStack() as ph:
                _ln(nc, tc, ph, x_t, t2_t, c_invd_r, c_eps)
            # ---------- SA front (k/v proj + phi), CA proj interleaved
            safr = ExitStack()
            fr = safr.enter_context(tc.tile_pool(name="safr", bufs=1))
            kT_sa = fr.tile([128, KD, SL], bf16, name="sa_kT")
            with ExitStack() as ph:
                _proj_F(nc, tc, ph, t2_t, w_k, cb["sa_bk"], kT_sa)
            vaug_sa = fr.tile([128, RT, H, 65], bf16, name="sa_vaug")
            nc.vector.memset(vaug_sa[:, :, :, 64:65], 1.0)
            bvr_sa = fr.tile([128, D], bf16, name="sa_bvr")
            nc.gpsimd.partition_broadcast(bvr_sa[:], cb["sa_bv"][:],
                                          channels=128)
            with ExitStack() as ph:
                _proj_R_vaug(nc, tc, ph, t2_t, w_v, bvr_sa, vaug_sa)
            kvsb_sa = fr.tile([65, H, 258], bf16, name="sa_kvsb")

            # CA front tiles + weights prepped now; its projection units
            # run as fillers inside the SA phi pipeline
            cafr = ExitStack()
            cfr = cafr.enter_context(tc.tile_pool(name="cafr", bufs=1,
                                                  side="right"))
            ca_ps_stack = ExitStack()
            ca_ps_k = ca_ps_stack.enter_context(
                tc.tile_pool(name="capfps", bufs=1, space="PSUM"))
            ca_ps_v = ca_ps_stack.enter_context(
                tc.tile_pool(name="capvps", bufs=1, space="PSUM"))
            w_k_ca = _load_w(nc, wp, wslice("ca_wk"))
            w_v_ca = _load_w(nc, wp, wslice("ca_wv"))
            kT_ca = cfr.tile([128, KD, SL], bf16, name="ca_kT")
            vaug_ca = cfr.tile([128, RT, H, 65], bf16, name="ca_vaug")
            nc.vector.memset(vaug_ca[:, :, :, 64:65], 1.0)
            bvr_ca = cfr.tile([128, D], bf16, name="ca_bvr")
            nc.gpsimd.partition_broadcast(bvr_ca[:], cb["ca_bv"][:],
                                          channels=128)
            kvsb_ca = cfr.tile([65, H, 258], bf16, name="ca_kvsb")
            ca_units = (
                [lambda m=m: _proj_F_unit(nc, ca_ps_k, mem_t, w_k_ca,
                                          cb["ca_bk"], kT_ca, m,
                                          dve_evac=True)
                 for m in range(KD)] +
                [lambda rt=rt: _proj_R_unit(nc, ca_ps_v, mem_t, w_v_ca,
                                            bvr_ca, vaug_ca, rt)
                 for rt in range(RT)])

            def sa_half_ar(i):
                half = "ab"[i]
                hs = slice(8 * i, 8 * i + 8)
                nc.sync.dma_start(out=ccs["sa_kv_in_" + half][:],
                                  in_=kvsb_sa[0:65, hs, :])
                if no_ar:
                    nc.sync.dma_start(out=ccs["sa_kv_out_" + half][:],
                                      in_=ccs["sa_kv_in_" + half][:])
                else:
                    nc.gpsimd.collective_compute(
                        "AllReduce", ALU.add, replica_groups=RG,
                        ins=[ccs["sa_kv_in_" + half].opt()],
                        outs=[ccs["sa_kv_out_" + half].opt()])

            with ExitStack() as ph:
                _phi_k_kv(nc, tc, ph, kT_sa, vaug_sa, cb["sa_wf"],
                          c_negblk, c_b64, kvsb_sa, fillers=ca_units,
                          half_cb=sa_half_ar)
            safr.close()
            ca_ps_stack.close()

            # ---------- CA phi; SA q-proj interleaved (covers SA AR)
            qsap = top.enter_context(tc.tile_pool(name="qsap", bufs=1))
            qT_sa = qsap.tile([128, KD, SL], bf16, name="qT_sa")
            qps = cafr.enter_context(
                tc.tile_pool(name="qpfps", bufs=1, space="PSUM"))
            w_q = _load_w(nc, wp, wslice("sa_wq"))
            q_units = [lambda m=m: _proj_F_unit(nc, qps, t2_t, w_q,
                                                cb["sa_bq"], qT_sa, m,
                                                dve_evac=True)
                       for m in range(KD)]
            with ExitStack() as ph:
                _phi_k_kv(nc, tc, ph, kT_ca, vaug_ca, cb["ca_wf"],
                          c_negblk, c_b64, kvsb_ca, fillers=q_units,
                          kvbufs=2, pjbufs=3)
            launch_ar("ca", kvsb_ca)
            cafr.close()
            memstack.close()

            # ---------- SA back
            sabk = ExitStack()
            bk = sabk.enter_context(tc.tile_pool(name="sabk", bufs=1))
            w_o = _load_w(nc, wp, wslice("sa_wo"))
            kvT_sa = bk.tile([128, H, 2, 65], bf16, name="sa_kvT")
            kvcolT_sa = bk.tile([1, H, 65], bf16, name="sa_kvcolT")
            attn_sa = bk.tile([128, KD, SL], bf16, name="sa_attn")
            for i in range(2):
                with ExitStack() as ph:
                    _kv_consume(nc, tc, ph, ccs["sa_kv_out_" + "ab"[i]],
                                ident_bf, kvT_sa, kvcolT_sa, h0=8 * i, nh=8)
                with ExitStack() as ph:
                    _phi_q_out(nc, tc, ph, qT_sa, kvT_sa, kvcolT_sa,
                               cb["sa_wf"], c_negblk, ident_bf, c_lneps,
                               attn_sa, pairs=range(4 * i, 4 * i + 4))
            with ExitStack() as ph:
                _proj_add(nc, tc, ph, attn_sa, w_o, cb["sa_bo"], x_t)
            sabk.close()

            # ---------- CA back: consume AR early, then LN2 + q proj
            cabk = ExitStack()
            cbk = cabk.enter_context(tc.tile_pool(name="cabk", bufs=1))
            kvT_ca = cbk.tile([128, H, 2, 65], bf16, name="ca_kvT")
            kvcolT_ca = cbk.tile([1, H, 65], bf16, name="ca_kvcolT")
            with ExitStack() as ph:
                _kv_consume(nc, tc, ph, ccs["ca_kv_out"], ident_bf, kvT_ca,
                            kvcolT_ca)
            with ExitStack() as ph:
                _ln(nc, tc, ph, x_t, t2_t, c_invd_r, c_eps)
            w_q2 = _load_w(nc, wp, wslice("ca_wq"))
            qT_ca = cbk.tile([128, KD, SL], bf16, name="qT_ca")
            with ExitStack() as ph:
                _proj_F(nc, tc, ph, t2_t, w_q2, cb["ca_bq"], qT_ca)
            w_o2 = _load_w(nc, wp, wslice("ca_wo"))
            attn_ca = cbk.tile([128, KD, SL], bf16, name="ca_attn")
            with ExitStack() as ph:
                _phi_q_out(nc, tc, ph, qT_ca, kvT_ca, kvcolT_ca,
                           cb["ca_wf"], c_negblk, ident_bf, c_lneps,
                           attn_ca)
            with ExitStack() as ph:
                _proj_add(nc, tc, ph, attn_ca, w_o2, cb["ca_bo"], x_t)
            cabk.close()

            # ---------- LN3 + FFN (4 quarters of F) + residual in x_t
            w1 = _bslice(blob, "w1").rearrange("p (q k d) -> p q k d", q=4,
                                               k=KD)
            w2 = _bslice(blob, "w2").rearrange("p (q k d) -> p q k d", q=4,
                                               k=KD)
            with ExitStack() as ph:
                sb = ph.enter_context(tc.tile_pool(name="ffsb", bufs=3))
                wfp = ph.enter_context(tc.tile_pool(name="ffwp", bufs=2))
                h1p = ph.enter_context(tc.tile_pool(name="h1p", bufs=1))
                # prefetch quarter-0 weights so the DMAs ride under LN3
                w1q0 = wfp.tile([128, KD, 1024], bf16, tag="wffn",
                                name="w1q0")
                nc.sync.dma_start(out=w1q0[:], in_=w1[:, 0, :, :])
                w2q0 = wfp.tile([128, KD, 1024], bf16, tag="wffn",
                                name="w2q0")
                nc.sync.dma_start(out=w2q0[:], in_=w2[:, 0, :, :])
                with ExitStack() as lnph:
                    _ln(nc, tc, lnph, x_t, t2_t, c_invd_r, c_eps)
                ps1 = ph.enter_context(tc.tile_pool(name="f1ps", bufs=2,
                                                    space="PSUM"))
                ps2 = ph.enter_context(tc.tile_pool(name="f2ps", bufs=2,
                                                    space="PSUM"))
                for q in range(4):
                    if q == 0:
                        w1q, w2q = w1q0, w2q0
                    else:
                        w1q = wfp.tile([128, KD, 1024], bf16, tag="wffn",
                                       name="w1q")
                        nc.sync.dma_start(out=w1q[:], in_=w1[:, q, :, :])
                        w2q = wfp.tile([128, KD, 1024], bf16, tag="wffn",
                                       name="w2q")
                        nc.sync.dma_start(out=w2q[:], in_=w2[:, q, :, :])
                    h1 = h1p.tile([128, KD, SL], bf16, tag="h1", name="h1")
                    for m in range(KD):
                        o_ps = ps1.tile([128, SL], f32, tag="f1", name="f1")
                        for kd in range(KD):
                            for ch in range(NCH):
                                cs = bass.ts(ch, 512)
                                nc.tensor.matmul(
                                    o_ps[:, cs],
                                    w1q[:, kd, m * 128:(m + 1) * 128],
                                    t2_t[:, kd, cs],
                                    start=(kd == 0), stop=(kd == KD - 1),
                                    skip_group_check=True)
                        nc.scalar.activation(
                            h1[:, m, :], o_ps[:], AF.Relu,
                            bias=cb["b1"][:, q * 8 + m:q * 8 + m + 1])
                    for m in range(KD):
                        o_ps = ps2.tile([128, SL], f32, tag="f2", name="f2")
                        for kf in range(KD):
                            for ch in range(NCH):
                                cs = bass.ts(ch, 512)
                                nc.tensor.matmul(
                                    o_ps[:, cs],
                                    w2q[:, kf, m * 128:(m + 1) * 128],
                                    h1[:, kf, cs],
                                    start=(kf == 0), stop=(kf == KD - 1),
                                    skip_group_check=True)
                        if q == 0:
                            nc.vector.scalar_tensor_tensor(
                                x_t[:, m, :], o_ps[:],
                                cb["b2"][:, m:m + 1], x_t[:, m, :],
                                ALU.add, ALU.add)
                        else:
                            nc.any.tensor_add(x_t[:, m, :],
                                              x_t[:, m, :], o_ps[:])
                            if q == 3:
                                # stream the finished m-tile out
                                nc.sync.dma_start(
                                    out=outT[:, m, :],
                                    in_=x_t[:, m, :].bitcast(f32))
    nc.finalize()
    return nc


# ------------------------------------------------------------------ host

def _prep_inputs(inputs):
    Cs = DH ** -0.25
    f = np.float32
    bf = ml_dtypes.bfloat16
    inp = {k: np.asarray(v, dtype=f) for k, v in inputs.items()}

    def fshape(vec):
        n = vec.shape[0] // 128
        return np.ascontiguousarray(vec.reshape(n, 128).T)

    def wpack(w_t):
        # (din, dout) -> (128, KD, dout)
        dout = w_t.shape[1]
        return np.ascontiguousarray(
            w_t.reshape(KD, 128, dout).transpose(1, 0, 2))

    def cols(arr):
        """(128, ...) array of f32/bf16 -> (128, n) bf16 byte view."""
        a = np.ascontiguousarray(arr)
        a = a.reshape(128, -1)
        return a.view(bf)

    shared = {}
    shared["c_ident"] = np.eye(128, dtype=bf)

    # fold LN gamma/beta into the consumers of each LN output
    ln_fold = {"sa_wq": "1", "sa_wk": "1", "sa_wv": "1", "ca_wq": "2"}
    bias_of = {"wq": "bq", "wk": "bk", "wv": "bv"}
    biases = {p + "_" + b: inp[p + "_" + b].copy()
              for p in ("sa", "ca") for b in ("bq", "bk", "bv", "bo")}
    for pre in ("sa", "ca"):
        for nm in ("wq", "wk", "wv", "wo"):
            key = pre + "_" + nm
            w_t = np.ascontiguousarray(inp[key].T)
            if key in ln_fold:
                i = ln_fold[key]
                w_t = w_t * inp["ln%s_g" % i][:, None]
                biases[pre + "_" + bias_of[nm]] += (
                    inp[key] @ inp["ln%s_b" % i])
            shared[key] = wpack(w_t).astype(bf)
        for nm in ("bq", "bk", "bo"):
            shared[pre + "_" + nm] = fshape(biases[pre + "_" + nm])
        bvrow = np.zeros((128, D), bf)
        bvrow[0, :] = biases[pre + "_bv"].astype(bf)
        shared[pre + "_bv"] = bvrow
        wf_t = (Cs * inp[pre + "_feat"]).T          # (DH, M)
        wfc = np.zeros((128, 512), f)
        wfc[0:64, 0:256] = wf_t
        wfc[64:128, 256:512] = wf_t
        shared[pre + "_wf"] = wfc.astype(bf)
    a = np.ascontiguousarray(inp["ff_w1"].T) * inp["ln3_g"][:, None]
    b1_fold = inp["ff_b1"] + inp["ff_w1"] @ inp["ln3_b"]
    shared["w1"] = np.ascontiguousarray(
        a.reshape(KD, 128, 4, 1024).transpose(1, 2, 0, 3)).astype(bf)
    b = np.ascontiguousarray(inp["ff_w2"].T)            # (F, D)
    shared["w2"] = np.ascontiguousarray(
        b.reshape(4, KD, 128, 1024).transpose(2, 0, 1, 3)).astype(bf)
    shared["b1"] = fshape(b1_fold)
    shared["b2"] = fshape(inp["ff_b2"])

    shared_cols = {nm: cols(shared[nm]) for nm in shared}

    in_maps = []
    for core in range(8):
        b_ix, half = core // 2, core % 2
        sl = slice(half * SL, (half + 1) * SL)
        xt = np.ascontiguousarray(inp["tgt"][sl, b_ix, :].T)      # (D, SL)
        xT = np.ascontiguousarray(
            xt.reshape(KD, 128, SL).transpose(1, 0, 2))
        mt = np.ascontiguousarray(inp["memory"][sl, b_ix, :].T)
        memT = np.ascontiguousarray(
            mt.reshape(KD, 128, SL).transpose(1, 0, 2)).astype(bf)
        percore = {"xT": cols(xT), "memT": cols(memT)}
        blocks = []
        for nm, ncols in _BLOB_SPEC:
            blk = percore.get(nm)
            if blk is None:
                blk = shared_cols[nm]
            assert blk.shape == (128, ncols), (nm, blk.shape, ncols)
            blocks.append(blk)
        in_maps.append({"blob": np.concatenate(blocks, axis=1)})
    return in_maps


def _build_exec(nc, n_cores=8):
    import jax
    from jax.sharding import Mesh, PartitionSpec
    from jax.experimental.shard_map import shard_map
    from concourse import bass2jax as b2j

    b2j.install_neuronx_cc_hook()
    partition_name = (nc.partition_id_tensor.name
                      if nc.partition_id_tensor else None)
    in_names, out_names, out_avals = [], [], []
    for alloc in nc.m.functions[0].allocations:
        if not isinstance(alloc, mybir.MemoryLocationSet):
            continue
        name = alloc.memorylocations[0].name
        if alloc.kind == "ExternalInput":
            if name != partition_name:
                in_names.append(name)
        elif alloc.kind == "ExternalOutput":
            out_names.append(name)
            out_avals.append(jax.core.ShapedArray(
                tuple(alloc.tensor_shape), mybir.dt.np(alloc.dtype)))
    n_params = len(in_names)
    all_in = list(in_names) + list(out_names)
    if partition_name is not None:
        all_in.append(partition_name)

    def _body(*args):
        operands = list(args)
        if partition_name is not None:
            operands.append(b2j.partition_id_tensor())
        outs = b2j._bass_exec_p.bind(
            *operands, out_avals=tuple(out_avals), in_names=tuple(all_in),
            out_names=tuple(out_names), lowering_input_output_aliases=(),
            sim_require_finite=True, sim_require_nnan=True, nc=nc)
        return tuple(outs)

    devices = jax.devices()[:n_cores]
    mesh = Mesh(np.asarray(devices), ("core",))
    n_outs = len(out_names)
    specs = (PartitionSpec("core"),) * (n_params + n_outs)
    out_specs = (PartitionSpec("core"),) * n_outs
    donate = tuple(range(n_params, n_params + n_outs))
    sharded = jax.jit(shard_map(_body, mesh=mesh, in_specs=specs,
                                out_specs=out_specs, check_rep=False),
                      donate_argnums=donate, keep_unused=True)

    def run(in_maps, fetch=True):
        import jax as _jax
        concat = [np.concatenate([np.asarray(in_maps[c][nm])
                                  for c in range(n_cores)], axis=0)
                  for nm in in_names]
        zeros = [np.zeros((n_cores * av.shape[0], *av.shape[1:]), av.dtype)
                 for av in out_avals]
        outs = sharded(*concat, *zeros)
        if not fetch:
            _jax.block_until_ready(outs)
            return None
        return [{nm: np.asarray(outs[i]).reshape(
            n_cores, *out_avals[i].shape)[c]
            for i, nm in enumerate(out_names)} for c in range(n_cores)]

    def time_exec(in_maps, iters=8):
        """Wall-time the sharded exec with device-resident inputs."""
        import time as _time
        import jax as _jax
        from jax.sharding import NamedSharding
        sh = NamedSharding(mesh, PartitionSpec("core"))
        concat = [np.concatenate([np.asarray(in_maps[c][nm])
                                  for c in range(n_cores)], axis=0)
                  for nm in in_names]
        dev_in = _jax.device_put(concat, [sh] * len(concat))
        _jax.block_until_ready(dev_in)
        zeros = [np.zeros((n_cores * av.shape[0], *av.shape[1:]), av.dtype)
                 for av in out_avals]
        times = []
        for _ in range(iters):
            zd = _jax.device_put(zeros, [sh] * len(zeros))
            _jax.block_until_ready(zd)
            t0 = _time.time()
            outs = sharded(*dev_in, *zd)
            _jax.block_until_ready(outs)
            times.append(_time.time() - t0)
        return times

    run.in_names = in_names
    run.time_exec = time_exec
    run.sharded = sharded
    run.mesh = mesh
    run.out_avals = out_avals
    run.n_params = n_params
    return run


def _get_exec():
    if "exec" not in _CACHE:
        nc = build_nc()
        _CACHE["exec"] = _build_exec(nc)
    return _CACHE["exec"]


def kernel(**inputs):
    run = _get_exec()
    in_maps = _prep_inputs(inputs)
    res = run(in_maps)
    out = np.empty((S, B, D), np.float32)
    for c in range(8):
        b_ix, half = c // 2, c % 2
        slab = res[c]["outT"]                       # (128, KD, SL)
        out[half * SL:(half + 1) * SL, b_ix, :] = (
            slab.transpose(1, 0, 2).reshape(D, SL).T)
    return out


# revision 28
# speedup vs baseline: 1.0207x; 1.0207x over previous
"""Trainium2 Bass kernel for nn_CustomDecoderLayer (FAVOR+ decoder layer).

Sharding: 8 cores = 4 batches x 2 sequence halves (S'=1024 tokens/core),
full D/heads/F per core. The only collectives are one bf16 AllReduce-add
of the FAVOR+ kv summary (65 x H*258, ~0.5 MB) per attention block,
within each core pair; column 257 carries exp(8*(local_stab-8)) so the
same AllReduce transports the k-stabilizer (recovered via x^(-1/8)).
Both ARs are launched early and covered by independent compute (the
cross-attn k/v path does not depend on self-attn output; its projection
units run as fillers inside the self-attn phi(k) pipeline). The
residual stays in SBUF for the whole kernel; LayerNorm gamma/beta are
folded into the downstream projection weights host-side.

All inputs ride in ONE host-packed bf16 blob per core (f32 slices are
bitcast views) -- per-exec dispatch overhead scales with the input
buffer count (~20us/buffer), so 30 tensors -> 1 saves ~0.6 ms/exec.
"""
import sys
sys.path.insert(0, "/opt/trn_rl_repo")
from contextlib import ExitStack

import numpy as np
import ml_dtypes

import concourse.bass as bass
import concourse.mybir as mybir
import concourse.tile as tile
from concourse import bacc, bass_isa

f32 = mybir.dt.float32
f32r = mybir.dt.float32r
bf16 = mybir.dt.bfloat16
AF = mybir.ActivationFunctionType
AX = mybir.AxisListType
ALU = mybir.AluOpType

D, H, DH, M = 1024, 16, 64, 256
S, B, F = 2048, 4, 4096
SL = 1024                     # tokens per core (one seq half)
KD = D // 128                 # 8
RT = SL // 128                # 8
NCH = SL // 512               # 2
NP = H // 2                   # 8 head pairs
C2 = 0.5 * (DH ** -0.5)       # 0.0625, exact in bf16
EPS16 = 1.0e-6 * 16.0
RG = [[0, 1], [2, 3], [4, 5], [6, 7]]

_CACHE = {}

# ---------------------------------------------------------------- blob
# One bf16 dram tensor per core; (name, bf16-cols, note). f32 payloads
# occupy 2 cols per value and are bitcast device-side.

_BLOB_SPEC = [
    ("xT", 2 * KD * SL),        # f32 (128, KD, SL)
    ("sa_wk", KD * D),          # bf16 (128, KD, D)
    ("sa_bk", 2 * KD),          # f32 (128, KD)
    ("sa_wf", 512),             # bf16 (128, 512)
    ("sa_wv", KD * D),
    ("sa_bv", D),               # bf16 (1, D) in partition 0
    ("memT", KD * SL),          # bf16 (128, KD, SL)
    ("ca_wk", KD * D),
    ("ca_bk", 2 * KD),
    ("ca_wf", 512),
    ("ca_wv", KD * D),
    ("ca_bv", D),
    ("sa_wq", KD * D),
    ("sa_bq", 2 * KD),
    ("c_ident", 128),           # bf16 (128, 128)
    ("sa_wo", KD * D),
    ("sa_bo", 2 * KD),
    ("ca_wq", KD * D),
    ("ca_bq", 2 * KD),
    ("ca_wo", KD * D),
    ("ca_bo", 2 * KD),
    ("w1", 4 * KD * 1024),      # bf16 (128, 4, KD, 1024)
    ("b1", 2 * 32),             # f32 (128, 32)
    ("w2", 4 * KD * 1024),
    ("b2", 2 * KD),             # f32 (128, KD)
]
_BLOB_OFF = {}
_off = 0
for _nm, _c in _BLOB_SPEC:
    _BLOB_OFF[_nm] = _off
    _off += _c
BLOB_COLS = _off


def _bslice(blob, name):
    off = _BLOB_OFF[name]
    cols = dict(_BLOB_SPEC)[name]
    return blob[:, off:off + cols]


def _ln(nc, tc, ctx, x_t, out_t, c_invd, c_eps):
    """LayerNorm F-layout: x_t (128, KD, SL) f32r -> out_t bf16.

    Stats for both chunks first, then normalize kd-outer so consumers
    that read t2 per-kd (full SL) unblock as early as possible. The
    affine gamma/beta are folded into downstream weights host-side, so
    this emits plain (x - mu) * rstd."""
    ps = ctx.enter_context(tc.tile_pool(name="lnps", bufs=2, space="PSUM"))
    sb = ctx.enter_context(tc.tile_pool(name="lnsb", bufs=3))
    mus, rstds = [], []
    for ch in range(NCH):
        cs = bass.ts(ch, 512)
        mv = ps.tile([128, 2, 512], f32, tag="ln_ps", name="ln_ps")
        for kd in range(KD):
            x2 = sb.tile([128, 512], f32r, tag="ln_x2", name="ln_x2")
            nc.scalar.activation(x2[:], x_t[:, kd, cs].bitcast(f32),
                                 AF.Square)
            nc.tensor.matmul(mv[:, 0, :], c_invd[:], x_t[:, kd, cs],
                             start=(kd == 0), stop=(kd == KD - 1),
                             skip_group_check=True)
            nc.tensor.matmul(mv[:, 1, :], c_invd[:], x2[:],
                             start=(kd == 0), stop=(kd == KD - 1),
                             skip_group_check=True)
        mu = sb.tile([128, 512], f32, tag="ln_mu", name="ln_mu", bufs=2)
        nc.any.tensor_copy(mu[:], mv[:, 0, :])
        mu2 = sb.tile([128, 512], f32, tag="ln_mu2", name="ln_mu2")
        nc.vector.tensor_mul(mu2[:], mu[:], mu[:])
        var = sb.tile([128, 512], f32, tag="ln_var", name="ln_var")
        nc.vector.tensor_sub(var[:], mv[:, 1, :], mu2[:])
        sd = sb.tile([128, 512], f32, tag="ln_sd", name="ln_sd")
        nc.scalar.activation(sd[:], var[:], AF.Sqrt, bias=c_eps[:])
        rstd = sb.tile([128, 512], f32, tag="ln_rstd", name="ln_rstd")
        nc.vector.reciprocal(rstd[:], sd[:])
        mus.append(mu)
        rstds.append(rstd)
    for kd in range(KD):
        for ch in range(NCH):
            cs = bass.ts(ch, 512)
            # split normalize units across DVE and Pool (~10:6)
            eng = nc.vector if (kd * NCH + ch) % 8 < 5 else nc.gpsimd
            xm = sb.tile([128, 512], f32, tag="ln_xm", name="ln_xm")
            eng.tensor_sub(xm[:], x_t[:, kd, cs], mus[ch][:])
            eng.tensor_mul(out_t[:, kd, cs], xm[:], rstds[ch][:])


def _load_w(nc, wp, w_ap):
    """One-DMA load of a host-packed (128, KD, 1024) bf16 weight."""
    w_sb = wp.tile([128, KD, D], bf16, tag="w", name="w_sb")
    nc.sync.dma_start(out=w_sb[:], in_=w_ap)
    return w_sb


def _proj_F_unit(nc, ps, src_t, w_sb, b_t, out_t, m, dve_evac=False):
    """One m-tile of an F-layout projection."""
    o_ps = ps.tile([128, SL], f32, tag="proj_ps", name="proj_ps")
    for kd in range(KD):
        for ch in range(NCH):
            cs = bass.ts(ch, 512)
            nc.tensor.matmul(o_ps[:, cs], w_sb[:, kd, m * 128:(m + 1) * 128],
                             src_t[:, kd, cs],
                             start=(kd == 0), stop=(kd == KD - 1),
                             skip_group_check=True)
    nc.any.tensor_scalar_add(out_t[:, m, :], o_ps[:], b_t[:, m:m + 1])


def _proj_F(nc, tc, ctx, src_t, w_sb, b_t, out_t):
    """F-layout projection: out[dout, tok]. src (128, KD, SL) bf16,
    w_sb (128, KD, D) [din, dout]. Full-SL moving operand (bf16)."""
    ps = ctx.enter_context(tc.tile_pool(name="pfps", bufs=2, space="PSUM"))
    for m in range(KD):
        _proj_F_unit(nc, ps, src_t, w_sb, b_t, out_t, m)


def _proj_R_unit(nc, ps, src_t, w_sb, bv_rep, v_aug, rt):
    """One token-tile of the R-layout V projection."""
    rs = bass.ts(rt, 128)
    v_ps = ps.tile([128, SL], f32, tag="v_ps", name="v_ps")
    for kd in range(KD):
        for ch in range(NCH):
            cs = bass.ts(ch, 512)
            nc.tensor.matmul(v_ps[:, cs], src_t[:, kd, rs],
                             w_sb[:, kd, cs],
                             start=(kd == 0), stop=(kd == KD - 1),
                             skip_group_check=True)
    nc.any.tensor_add(v_aug[:, rt, :, 0:64], v_ps[:], bv_rep[:])


def _proj_R_vaug(nc, tc, ctx, src_t, w_sb, bv_rep, v_aug):
    """R-layout V projection into v_aug (128, RT, H, 65) bf16."""
    ps = ctx.enter_context(tc.tile_pool(name="pvps", bufs=2, space="PSUM"))
    for rt in range(RT):
        _proj_R_unit(nc, ps, src_t, w_sb, bv_rep, v_aug, rt)


def _phi_k_kv(nc, tc, ctx, kT, v_aug, wf_cat, c_negblk, c_b64, kv_sb,
              fillers=(), half_cb=None, ebufs=2, kvbufs=1, pjbufs=2):
    """phi(k) + local kv partials, all heads. E carries no stabilizer;
    row 65 of kv_sb gets exp(8*(local_stab - 8)) so the single
    AllReduce-add also transports the stab: the consume side recovers
    ~exp(-max_stab) as (sum)^(-1/8) * e^-8 (error <= log2/8 in the
    exponent, which only perturbs the eps weighting by <1%).

    `fillers` are independent PE-dense work units (closures) interleaved
    between pipeline stages; each pair's kv matmuls are emitted one pair
    late so the in-order PE queue never waits on the Act-engine exps."""
    ps_pj = ctx.enter_context(tc.tile_pool(name="pkpj", bufs=pjbufs,
                                           space="PSUM"))
    ps_ns = ctx.enter_context(tc.tile_pool(name="pkns", bufs=1, space="PSUM"))
    ps_kv = ctx.enter_context(tc.tile_pool(name="pkkv", bufs=kvbufs,
                                           space="PSUM"))
    sb = ctx.enter_context(tc.tile_pool(name="pksb", bufs=3))
    ep = ctx.enter_context(tc.tile_pool(name="pkep", bufs=ebufs))
    nc.vector.memset(kv_sb[:, :, 257:258], 0.0)
    fill_iter = iter(fillers)

    def emit_fill(n):
        for _ in range(n):
            f = next(fill_iter, None)
            if f is not None:
                f()

    def stage_kv(g, E_t):
        for h in range(2):
            kv_ps = ps_kv.tile([65, 257], f32, tag="kv_ps", name="kv_ps")
            for rt in range(RT):
                nc.tensor.matmul(kv_ps[:], v_aug[:, rt, 2 * g + h, :],
                                 E_t[:, rt, h, :], start=(rt == 0),
                                 stop=(rt == RT - 1), skip_group_check=True)
            nc.any.tensor_copy(kv_sb[0:65, 2 * g + h, 0:257], kv_ps[:])

    prev = None
    for g in range(NP):
        E_t = ep.tile([128, RT, 2, 257], bf16, tag="E_t", name="E_t")
        nc.vector.memset(E_t[:, :, :, 256:257], EPS16)
        nsq_all = sb.tile([128, RT, 2], f32, tag="nsq_all", name="nsq_all")
        for rt in range(RT):
            rs = bass.ts(rt, 128)
            k2 = sb.tile([128, 128], bf16, tag="k2", name="k2")
            nc.vector.tensor_mul(k2[:], kT[:, g, rs], kT[:, g, rs])
            pj = ps_pj.tile([128, 2, 256], f32, tag="pj", name="pj_k")
            nc.tensor.matmul(pj[:, :, :], kT[:, g, rs], wf_cat[:])
            nsq = ps_ns.tile([128, 2], f32, tag="nsq", name="nsq_k")
            nc.tensor.matmul(nsq[:], k2[:], c_negblk[:])
            nc.any.tensor_copy(nsq_all[:, rt, :], nsq[:])
            for h in range(2):
                nc.scalar.activation(E_t[:, rt, h, 0:256], pj[:, h, :],
                                     AF.Exp, bias=nsq_all[:, rt, h:h + 1])
        # stabilizer recovered post-exp: max_m E = e^(stab_tok - sq), so
        # stab_tok = ln(max_m E) + sq; one wide bf16 reduce replaces 8
        # PSUM f32 reduce_max ops.
        # transport value exp(8*(stab-8)) = maxE^8 * e^(8*sq-64), computed
        # without Ln (avoids Act table reloads between the Exps): maxE =
        # e^(stab_tok - sq), cube-squared on DVE; the e^(8*sq-64) factor is
        # an Exp (same Act table as phi).
        maxE = sb.tile([128, RT, 2], f32, tag="maxE", name="maxE")
        nc.vector.reduce_max(maxE[:], E_t[:, :, :, 0:256], axis=AX.X)
        m2 = sb.tile([128, RT, 2], f32, tag="m2", name="m2")
        nc.vector.tensor_mul(m2[:], maxE[:], maxE[:])
        nc.vector.tensor_mul(m2[:], m2[:], m2[:])
        nc.vector.tensor_mul(m2[:], m2[:], m2[:])
        e8 = sb.tile([128, RT, 2], f32, tag="e8", name="e8")
        nc.scalar.activation(e8[:], nsq_all[:], AF.Exp, scale=-8.0,
                             bias=c_b64[:])
        tv = sb.tile([128, RT, 2], f32, tag="tv", name="tv")
        nc.vector.tensor_mul(tv[:], m2[:], e8[:])
        stab_run = sb.tile([128, 2], f32, tag="stab_run", name="stab_run")
        nc.vector.reduce_max(stab_run[:],
                             tv[:].rearrange("p r h -> p h r"),
                             axis=AX.X)
        stab_rep = sb.tile([128, 2], f32, tag="stab_rep", name="stab_rep")
        nc.gpsimd.partition_all_reduce(stab_rep[:], stab_run[:], channels=128,
                                       reduce_op=bass_isa.ReduceOp.max)
        nc.any.tensor_copy(kv_sb[0:1, 2 * g:2 * g + 2, 257:258],
                           stab_rep[0:1, :])
        emit_fill(2)
        if prev is not None:
            stage_kv(*prev)
            if prev[0] == 3 and half_cb is not None:
                half_cb(0)
        prev = (g, E_t)
    stage_kv(*prev)
    if half_cb is not None:
        half_cb(1)
    emit_fill(len(fillers))


def _kv_consume(nc, tc, ctx, kv_out, ident_bf, kvT, kvcolT, h0=0, nh=H):
    """Load AR result; recover s = ~exp(-stab_max) from the summed
    exp(8*(stab-8)) row via three chained sqrts + fast reciprocal;
    scale kv by s and fold in the (unscaled) eps column; transpose
    -> kvT. Also emits kvcolT[1, h, 65] = sum_m kva (the rank-1
    stationary for the exact q-side eps correction)."""
    sb = ctx.enter_context(tc.tile_pool(name="kcsb", bufs=2))
    kvp = ctx.enter_context(tc.tile_pool(name="kckv", bufs=1))
    ps_tp = ctx.enter_context(tc.tile_pool(name="kctp", bufs=2, space="PSUM"))
    kv2 = kvp.tile([65, nh, 258], bf16, name="kv2")
    nc.sync.dma_start(out=kv2[:], in_=kv_out[:])
    s_row = sb.tile([1, nh], f32, tag="s_row", name="s_row")
    nc.any.tensor_copy(s_row[:], kv2[0:1, :, 257:258])
    r1 = sb.tile([1, nh], f32, tag="r1", name="r1")
    nc.scalar.activation(r1[:], s_row[:], AF.Sqrt)
    r2 = sb.tile([1, nh], f32, tag="r2", name="r2")
    nc.scalar.activation(r2[:], r1[:], AF.Sqrt)
    r3 = sb.tile([1, nh], f32, tag="r3", name="r3")
    nc.scalar.activation(r3[:], r2[:], AF.Sqrt)
    r4 = sb.tile([1, nh], f32, tag="r4", name="r4")
    nc.vector.reciprocal(r4[:], r3[:])
    s_t = sb.tile([1, nh], f32, tag="s_t", name="s_t")
    nc.vector.tensor_scalar_mul(s_t[:], r4[:], float(np.exp(-8.0)))
    s_bc = sb.tile([128, nh], f32, tag="s_bc", name="s_bc")
    nc.gpsimd.partition_broadcast(s_bc[:], s_t[:], channels=128)
    for h in range(nh):
        csum = sb.tile([65, 1], f32, tag="csum", name="csum")
        nc.any.tensor_copy(csum[:], kv2[0:65, h, 256:257])
        kva = sb.tile([65, 256], bf16, tag="kva", name="kva")
        nc.vector.tensor_scalar(kva[:], kv2[0:65, h, 0:256],
                                s_bc[0:65, h:h + 1], csum[:],
                                ALU.mult, ALU.add)
        kvcol = sb.tile([65, 1], f32, tag="kvcol", name="kvcol")
        nc.vector.reduce_sum(kvcol[:], kva[:], axis=AX.X)
        kvcolb = sb.tile([65, 1], bf16, tag="kvcolb", name="kvcolb")
        nc.any.tensor_copy(kvcolb[:], kvcol[:])
        tpc = ps_tp.tile([1, 65], bf16, tag="tp_kv", name="tp_kvc")
        nc.tensor.transpose(tpc[:], kvcolb[:], ident_bf[0:65, 0:65])
        nc.any.tensor_copy(kvcolT[0:1, h0 + h, :], tpc[:])
        for mt in range(2):
            tp = ps_tp.tile([128, 65], bf16, tag="tp_kv", name="tp_kv")
            nc.tensor.transpose(tp[:], kva[0:65, mt * 128:(mt + 1) * 128],
                                ident_bf[0:65, 0:65])
            nc.any.tensor_copy(kvT[:, h0 + h, mt, :], tp[:])


def _phi_q_out(nc, tc, ctx, qT, kvT, kvcolT, wf_cat, c_negblk, ident_bf,
               c_lneps, attn_t, pairs=range(NP)):
    """phi(q), exact reference semantics, stabilizer-free.

    A per-token scale on pq cancels exactly in out/z, so pq is used
    UNSCALED: pq_u = e^(proj - sq) (bf16-safe, <= e^5.3 here). The only
    place the reference stabilizer matters is the relative weight of
    its +eps term, which equals a rank-1 correction
    eps16 * e^(stab_tok) * colsum_m(kva) -- added exactly via a K=1
    matmul accumulated into the same PSUM group (e^(stab_tok) =
    rowmax(pq_u) * e^(sq), both cheap post-exp byproducts). This
    removes all per-rt PSUM reductions and the scale barrier from the
    q path."""
    ps_pj = ctx.enter_context(tc.tile_pool(name="pqpj", bufs=2, space="PSUM"))
    ps_ns = ctx.enter_context(tc.tile_pool(name="pqns", bufs=1, space="PSUM"))
    ps_tp = ctx.enter_context(tc.tile_pool(name="pqtp", bufs=3, space="PSUM"))
    ps_o = ctx.enter_context(tc.tile_pool(name="pqo", bufs=2, space="PSUM"))
    sb = ctx.enter_context(tc.tile_pool(name="pqsb", bufs=3))
    pqrp = ctx.enter_context(tc.tile_pool(name="pqrp", bufs=2))
    pqp = ctx.enter_context(tc.tile_pool(name="pqpq", bufs=2))
    for g in pairs:
        pqR = pqrp.tile([128, RT, 2, 256], bf16, tag="pqR", name="pqR")
        nsq_all = sb.tile([128, RT, 2], f32, tag="nsq_all", name="nsq_all")
        rmax = sb.tile([128, RT, 2], f32, tag="rmax_q", name="rmax_q")
        enq = sb.tile([128, RT, 2], f32, tag="enq", name="enq")
        eff = sb.tile([128, RT, 2], bf16, tag="eff", name="eff")
        effT = sb.tile([1, 2, SL], bf16, tag="effT", name="effT", bufs=2)
        pqT = pqp.tile([128, 2, 2, SL], bf16, tag="pqT", name="pqT")
        for rt in range(RT):
            rs = bass.ts(rt, 128)
            q2 = sb.tile([128, 128], bf16, tag="q2", name="q2")
            nc.vector.tensor_mul(q2[:], qT[:, g, rs], qT[:, g, rs])
            pj = ps_pj.tile([128, 2, 256], f32, tag="pj", name="pj_q")
            nc.tensor.matmul(pj[:, :, :], qT[:, g, rs], wf_cat[:])
            nsq = ps_ns.tile([128, 2], f32, tag="nsq", name="nsq_q")
            nc.tensor.matmul(nsq[:], q2[:], c_negblk[:])
            nc.any.tensor_copy(nsq_all[:, rt, :], nsq[:])
            nc.scalar.activation(pqR[:, rt, :, :], pj[:, :, :], AF.Exp)
            tp = ps_tp.tile([128, 2, 2, 128], bf16, tag="tp_pq", name="tp_pq")
            for h in range(2):
                for mt in range(2):
                    nc.tensor.transpose(
                        tp[:, h, mt, :],
                        pqR[:, rt, h, mt * 128:(mt + 1) * 128], ident_bf[:])
            nc.any.tensor_copy(pqT[:, :, :, rs], tp[:])
            if rt % 4 == 3:
                # rank-1 eps factors for this 512-token half:
                # eff = eps16 * e^(stab_tok) = rowmax(pq_u) * eps16*e^(sq)
                hh = slice(rt - 3, rt + 1)
                nc.vector.reduce_max(rmax[:, hh, :], pqR[:, hh, :, :],
                                     axis=AX.X)
                nc.scalar.activation(enq[:, hh, :], nsq_all[:, hh, :],
                                     AF.Exp, scale=-1.0, bias=c_lneps[:])
                nc.vector.tensor_mul(eff[:, hh, :], rmax[:, hh, :],
                                     enq[:, hh, :])
                for rr in range(rt - 3, rt + 1):
                    tpe = ps_tp.tile([1, 2, 128], bf16, tag="tp_pq",
                                     name="tp_eff")
                    for h in range(2):
                        nc.tensor.transpose(tpe[:, h, :],
                                            eff[:, rr, h:h + 1],
                                            ident_bf[:])
                    nc.any.tensor_copy(effT[:, :, bass.ts(rr, 128)],
                                       tpe[:])
        for h in range(2):
            hp = slice(64 * h, 64 * h + 64)
            for ch in range(NCH):
                cs = bass.ts(ch, 512)
                o_ps = ps_o.tile([65, 512], f32, tag="o_ps", name="o_ps")
                for mt in range(2):
                    nc.tensor.matmul(o_ps[:], kvT[:, 2 * g + h, mt, :],
                                     pqT[:, h, mt, cs], start=(mt == 0),
                                     stop=False, skip_group_check=True)
                nc.tensor.matmul(o_ps[:], kvcolT[0:1, 2 * g + h, :],
                                 effT[0:1, h, cs], start=False, stop=True,
                                 skip_group_check=True)
                zr = sb.tile([1, 512], f32, tag="zr", name="zr")
                nc.vector.reciprocal(zr[:], o_ps[64:65, :])
                zb = sb.tile([64, 512], f32, tag="zb", name="zb", bufs=2)
                nc.gpsimd.partition_broadcast(zb[:], zr[:], channels=64)
                nc.any.tensor_mul(attn_t[hp, g, cs], o_ps[0:64, :],
                                  zb[:])


def _proj_add(nc, tc, ctx, src_t, w_sb, b_t, x_t):
    """Wo-style projection (bf16 src); adds result into x_t (f32r)."""
    ps = ctx.enter_context(tc.tile_pool(name="waps", bufs=2, space="PSUM"))
    for m in range(KD):
        o_ps = ps.tile([128, SL], f32, tag="wa_ps", name="wa_ps")
        for kd in range(KD):
            for ch in range(NCH):
                cs = bass.ts(ch, 512)
                nc.tensor.matmul(o_ps[:, cs],
                                 w_sb[:, kd, m * 128:(m + 1) * 128],
                                 src_t[:, kd, cs],
                                 start=(kd == 0), stop=(kd == KD - 1),
                                 skip_group_check=True)
        nc.vector.scalar_tensor_tensor(
            x_t[:, m, :], o_ps[:], b_t[:, m:m + 1], x_t[:, m, :],
            ALU.add, ALU.add)


def build_nc(no_ar=False):
    nc = bacc.Bacc("TRN2", target_bir_lowering=False, debug=False,
                   num_devices=8)

    blob = nc.dram_tensor("blob", [128, BLOB_COLS], bf16,
                          kind="ExternalInput").ap()

    def wslice(name):
        return _bslice(blob, name).rearrange("p (k d) -> p k d", k=KD)

    def fslice(name, n):
        return _bslice(blob, name).bitcast(f32)

    outT = nc.dram_tensor("outT", [128, KD, SL], f32,
                          kind="ExternalOutput").ap()

    with tile.TileContext(nc) as tc:
        with ExitStack() as top:
            dram = top.enter_context(tc.tile_pool(name="dram", bufs=1,
                                                  space="DRAM"))
            ccs = {}
            for half in "ab":
                ccs["sa_kv_in_" + half] = dram.tile(
                    [65, 8 * 258], bf16, name="sa_kv_in_" + half)
                ccs["sa_kv_out_" + half] = dram.tile(
                    [65, 8 * 258], bf16, name="sa_kv_out_" + half)
            ccs["ca_kv_in"] = dram.tile([65, H * 258], bf16,
                                        name="ca_kv_in")
            ccs["ca_kv_out"] = dram.tile([65, H * 258], bf16,
                                         name="ca_kv_out")

            # persistent activations first so their DMAs lead the queue
            const = top.enter_context(tc.tile_pool(name="const", bufs=1))
            xp = top.enter_context(tc.tile_pool(name="xp", bufs=1))
            x_t = xp.tile([128, KD, SL], f32r, name="x_t")
            xT = _bslice(blob, "xT").bitcast(f32r).rearrange(
                "p (k s) -> p k s", k=KD)
            for kd in range(KD):
                nc.sync.dma_start(out=x_t[:, kd, :], in_=xT[:, kd, :])

            wp = top.enter_context(tc.tile_pool(name="wp", bufs=2))
            # SA front weights ride right behind x so the first
            # projection is never DMA-starved; memory comes after.
            w_k = _load_w(nc, wp, wslice("sa_wk"))
            cb = {}
            for pre in ("sa", "ca"):
                for nm in ("bq", "bk", "bo"):
                    key = pre + "_" + nm
                    t = const.tile([128, KD], f32, name=pre + nm)
                    if key == "sa_bk":
                        nc.sync.dma_start(out=t[:], in_=fslice(key, KD))
                    cb[key] = t
                t = const.tile([1, D], bf16, name=pre + "bv")
                cb[pre + "_bv"] = t
                wfc = const.tile([128, 512], bf16, name=pre + "wfc")
                cb[pre + "_wf"] = wfc
            nc.sync.dma_start(out=cb["sa_wf"][:], in_=_bslice(blob, "sa_wf"))
            w_v = _load_w(nc, wp, wslice("sa_wv"))
            nc.sync.dma_start(out=cb["sa_bv"][:],
                              in_=_bslice(blob, "sa_bv")[0:1, :])
            memstack = ExitStack()
            memp = memstack.enter_context(tc.tile_pool(name="memp", bufs=1,
                                                       side="right"))
            mem_t = memp.tile([128, KD, SL], bf16, name="mem_t")
            nc.sync.dma_start(out=mem_t[:],
                              in_=_bslice(blob, "memT").rearrange(
                                  "p (k s) -> p k s", k=KD))
            for key in ("sa_bq", "sa_bo", "ca_bq", "ca_bk", "ca_bo"):
                nc.sync.dma_start(out=cb[key][:], in_=fslice(key, KD))
            nc.sync.dma_start(out=cb["ca_wf"][:], in_=_bslice(blob, "ca_wf"))
            nc.sync.dma_start(out=cb["ca_bv"][:],
                              in_=_bslice(blob, "ca_bv")[0:1, :])

            # on-device constants (no DMA)
            c_invd = const.tile([128, 128], f32, name="c_invd")
            nc.vector.memset(c_invd[:], 1.0 / D)
            c_invd_r = c_invd[:].bitcast(f32r)
            c_negblk = const.tile([128, 2], bf16, name="c_negblk")
            nc.vector.memset(c_negblk[:], 0.0)
            nc.vector.memset(c_negblk[0:64, 0:1], -C2)
            nc.vector.memset(c_negblk[64:128, 1:2], -C2)
            ident_bf = const.tile([128, 128], bf16, name="ident_bf")
            nc.sync.dma_start(out=ident_bf[:], in_=_bslice(blob, "c_ident"))
            c_eps = const.tile([128, 1], f32, name="c_eps")
            nc.vector.memset(c_eps[:], 1.0e-5)
            c_lneps = const.tile([128, 1], f32, name="c_lneps")
            nc.vector.memset(c_lneps[:], float(np.log(EPS16)))
            c_b64 = const.tile([128, 1], f32, name="c_b64")
            nc.vector.memset(c_b64[:], -64.0)
            for nm, shp in (("b1", [128, 32]), ("b2", [128, KD])):
                t = const.tile(shp, f32, name=nm)
                nc.sync.dma_start(out=t[:], in_=fslice(nm, shp[1]))
                cb[nm] = t

            t2_t = xp.tile([128, KD, SL], bf16, name="t2_t")

            def launch_ar(pre, kv_sb):
                nc.sync.dma_start(out=ccs[pre + "_kv_in"][:], in_=kv_sb[:])
                if no_ar:
                    nc.sync.dma_start(out=ccs[pre + "_kv_out"][:],
                                      in_=ccs[pre + "_kv_in"][:])
                else:
                    nc.gpsimd.collective_compute(
                        "AllReduce", ALU.add, replica_groups=RG,
                        ins=[ccs[pre + "_kv_in"].opt()],
                        outs=[ccs[pre + "_kv_out"].opt()])

            # ---------- LN1 + SA front
            with ExitStack() as ph:
                _ln(nc, tc, ph, x_t, t2_t, c_invd_r, c_eps)
            # ---------- SA front (k/v proj + phi), CA proj interleaved
            safr = ExitStack()
            fr = safr.enter_context(tc.tile_pool(name="safr", bufs=1))
            kT_sa = fr.tile([128, KD, SL], bf16, name="sa_kT")
            with ExitStack() as ph:
                _proj_F(nc, tc, ph, t2_t, w_k, cb["sa_bk"], kT_sa)
            vaug_sa = fr.tile([128, RT, H, 65], bf16, name="sa_vaug")
            nc.vector.memset(vaug_sa[:, :, :, 64:65], 1.0)
            bvr_sa = fr.tile([128, D], bf16, name="sa_bvr")
            nc.gpsimd.partition_broadcast(bvr_sa[:], cb["sa_bv"][:],
                                          channels=128)
            with ExitStack() as ph:
                _proj_R_vaug(nc, tc, ph, t2_t, w_v, bvr_sa, vaug_sa)
            kvsb_sa = fr.tile([65, H, 258], bf16, name="sa_kvsb")

            # CA front tiles + weights prepped now; its projection units
            # run as fillers inside the SA phi pipeline
            cafr = ExitStack()
            cfr = cafr.enter_context(tc.tile_pool(name="cafr", bufs=1,
                                                  side="right"))
            ca_ps_stack = ExitStack()
            ca_ps_k = ca_ps_stack.enter_context(
                tc.tile_pool(name="capfps", bufs=1, space="PSUM"))
            ca_ps_v = ca_ps_stack.enter_context(
                tc.tile_pool(name="capvps", bufs=1, space="PSUM"))
            w_k_ca = _load_w(nc, wp, wslice("ca_wk"))
            w_v_ca = _load_w(nc, wp, wslice("ca_wv"))
            kT_ca = cfr.tile([128, KD, SL], bf16, name="ca_kT")
            vaug_ca = cfr.tile([128, RT, H, 65], bf16, name="ca_vaug")
            nc.vector.memset(vaug_ca[:, :, :, 64:65], 1.0)
            bvr_ca = cfr.tile([128, D], bf16, name="ca_bvr")
            nc.gpsimd.partition_broadcast(bvr_ca[:], cb["ca_bv"][:],
                                          channels=128)
            kvsb_ca = cfr.tile([65, H, 258], bf16, name="ca_kvsb")
            ca_units = (
                [lambda m=m: _proj_F_unit(nc, ca_ps_k, mem_t, w_k_ca,
                                          cb["ca_bk"], kT_ca, m,
                                          dve_evac=True)
                 for m in range(KD)] +
                [lambda rt=rt: _proj_R_unit(nc, ca_ps_v, mem_t, w_v_ca,
                                            bvr_ca, vaug_ca, rt)
                 for rt in range(RT)])

            def sa_half_ar(i):
                half = "ab"[i]
                hs = slice(8 * i, 8 * i + 8)
                nc.sync.dma_start(out=ccs["sa_kv_in_" + half][:],
                                  in_=kvsb_sa[0:65, hs, :])
                if no_ar:
                    nc.sync.dma_start(out=ccs["sa_kv_out_" + half][:],
                                      in_=ccs["sa_kv_in_" + half][:])
                else:
                    nc.gpsimd.collective_compute(
                        "AllReduce", ALU.add, replica_groups=RG,
                        ins=[ccs["sa_kv_in_" + half].opt()],
                        outs=[ccs["sa_kv_out_" + half].opt()])

            with ExitStack() as ph:
                _phi_k_kv(nc, tc, ph, kT_sa, vaug_sa, cb["sa_wf"],
                          c_negblk, c_b64, kvsb_sa, fillers=ca_units,
                          half_cb=sa_half_ar)
            safr.close()
            ca_ps_stack.close()

            # ---------- CA phi; SA q-proj interleaved (covers SA AR)
            qsap = top.enter_context(tc.tile_pool(name="qsap", bufs=1))
            qT_sa = qsap.tile([128, KD, SL], bf16, name="qT_sa")
            qps = cafr.enter_context(
                tc.tile_pool(name="qpfps", bufs=1, space="PSUM"))
            w_q = _load_w(nc, wp, wslice("sa_wq"))
            q_units = [lambda m=m: _proj_F_unit(nc, qps, t2_t, w_q,
                                                cb["sa_bq"], qT_sa, m,
                                                dve_evac=True)
                       for m in range(KD)]
            with ExitStack() as ph:
                _phi_k_kv(nc, tc, ph, kT_ca, vaug_ca, cb["ca_wf"],
                          c_negblk, c_b64, kvsb_ca, fillers=q_units,
                          kvbufs=2, pjbufs=3)
            launch_ar("ca", kvsb_ca)
            cafr.close()
            memstack.close()

            # ---------- SA back
            sabk = ExitStack()
            bk = sabk.enter_context(tc.tile_pool(name="sabk", bufs=1))
            w_o = _load_w(nc, wp, wslice("sa_wo"))
            kvT_sa = bk.tile([128, H, 2, 65], bf16, name="sa_kvT")
            kvcolT_sa = bk.tile([1, H, 65], bf16, name="sa_kvcolT")
            attn_sa = bk.tile([128, KD, SL], bf16, name="sa_attn")
            for i in range(2):
                with ExitStack() as ph:
                    _kv_consume(nc, tc, ph, ccs["sa_kv_out_" + "ab"[i]],
                                ident_bf, kvT_sa, kvcolT_sa, h0=8 * i, nh=8)
                with ExitStack() as ph:
                    _phi_q_out(nc, tc, ph, qT_sa, kvT_sa, kvcolT_sa,
                               cb["sa_wf"], c_negblk, ident_bf, c_lneps,
                               attn_sa, pairs=range(4 * i, 4 * i + 4))
            with ExitStack() as ph:
                _proj_add(nc, tc, ph, attn_sa, w_o, cb["sa_bo"], x_t)
            sabk.close()

            # ---------- CA back: consume AR early, then LN2 + q proj
            cabk = ExitStack()
            cbk = cabk.enter_context(tc.tile_pool(name="cabk", bufs=1))
            kvT_ca = cbk.tile([128, H, 2, 65], bf16, name="ca_kvT")
            kvcolT_ca = cbk.tile([1, H, 65], bf16, name="ca_kvcolT")
            with ExitStack() as ph:
                _kv_consume(nc, tc, ph, ccs["ca_kv_out"], ident_bf, kvT_ca,
                            kvcolT_ca)
            with ExitStack() as ph:
                _ln(nc, tc, ph, x_t, t2_t, c_invd_r, c_eps)
            w_q2 = _load_w(nc, wp, wslice("ca_wq"))
            qT_ca = cbk.tile([128, KD, SL], bf16, name="qT_ca")
            with ExitStack() as ph:
                _proj_F(nc, tc, ph, t2_t, w_q2, cb["ca_bq"], qT_ca)
            w_o2 = _load_w(nc, wp, wslice("ca_wo"))
            attn_ca = cbk.tile([128, KD, SL], bf16, name="ca_attn")
            with ExitStack() as ph:
                _phi_q_out(nc, tc, ph, qT_ca, kvT_ca, kvcolT_ca,
                           cb["ca_wf"], c_negblk, ident_bf, c_lneps,
                           attn_ca)
            with ExitStack() as ph:
                _proj_add(nc, tc, ph, attn_ca, w_o2, cb["ca_bo"], x_t)
            cabk.close()

            # ---------- LN3 + FFN (4 quarters of F) + residual in x_t
            w1 = _bslice(blob, "w1").rearrange("p (q k d) -> p q k d", q=4,
                                               k=KD)
            w2 = _bslice(blob, "w2").rearrange("p (q k d) -> p q k d", q=4,
                                               k=KD)
            with ExitStack() as ph:
                sb = ph.enter_context(tc.tile_pool(name="ffsb", bufs=3))
                wfp = ph.enter_context(tc.tile_pool(name="ffwp", bufs=2))
                h1p = ph.enter_context(tc.tile_pool(name="h1p", bufs=1))
                # prefetch quarter-0 weights so the DMAs ride under LN3
                w1q0 = wfp.tile([128, KD, 1024], bf16, tag="wffn",
                                name="w1q0")
                nc.sync.dma_start(out=w1q0[:], in_=w1[:, 0, :, :])
                w2q0 = wfp.tile([128, KD, 1024], bf16, tag="wffn",
                                name="w2q0")
                nc.sync.dma_start(out=w2q0[:], in_=w2[:, 0, :, :])
                with ExitStack() as lnph:
                    _ln(nc, tc, lnph, x_t, t2_t, c_invd_r, c_eps)
                ps1 = ph.enter_context(tc.tile_pool(name="f1ps", bufs=2,
                                                    space="PSUM"))
                ps2 = ph.enter_context(tc.tile_pool(name="f2ps", bufs=2,
                                                    space="PSUM"))
                for q in range(4):
                    if q == 0:
                        w1q, w2q = w1q0, w2q0
                    else:
                        w1q = wfp.tile([128, KD, 1024], bf16, tag="wffn",
                                       name="w1q")
                        nc.sync.dma_start(out=w1q[:], in_=w1[:, q, :, :])
                        w2q = wfp.tile([128, KD, 1024], bf16, tag="wffn",
                                       name="w2q")
                        nc.sync.dma_start(out=w2q[:], in_=w2[:, q, :, :])
                    h1 = h1p.tile([128, KD, SL], bf16, tag="h1", name="h1")
                    for m in range(KD):
                        o_ps = ps1.tile([128, SL], f32, tag="f1", name="f1")
                        for kd in range(KD):
                            for ch in range(NCH):
                                cs = bass.ts(ch, 512)
                                nc.tensor.matmul(
                                    o_ps[:, cs],
                                    w1q[:, kd, m * 128:(m + 1) * 128],
                                    t2_t[:, kd, cs],
                                    start=(kd == 0), stop=(kd == KD - 1),
                                    skip_group_check=True)
                        nc.scalar.activation(
                            h1[:, m, :], o_ps[:], AF.Relu,
                            bias=cb["b1"][:, q * 8 + m:q * 8 + m + 1])
                    for m in range(KD):
                        o_ps = ps2.tile([128, SL], f32, tag="f2", name="f2")
                        for kf in range(KD):
                            for ch in range(NCH):
                                cs = bass.ts(ch, 512)
                                nc.tensor.matmul(
                                    o_ps[:, cs],
                                    w2q[:, kf, m * 128:(m + 1) * 128],
                                    h1[:, kf, cs],
                                    start=(kf == 0), stop=(kf == KD - 1),
                                    skip_group_check=True)
                        if q == 0:
                            nc.vector.scalar_tensor_tensor(
                                x_t[:, m, :], o_ps[:],
                                cb["b2"][:, m:m + 1], x_t[:, m, :],
                                ALU.add, ALU.add)
                        else:
                            nc.any.tensor_add(x_t[:, m, :],
                                              x_t[:, m, :], o_ps[:])
                            if q == 3:
                                # stream the finished m-tile out
                                nc.sync.dma_start(
                                    out=outT[:, m, :],
                                    in_=x_t[:, m, :].bitcast(f32))
    nc.finalize()
    return nc


# ------------------------------------------------------------------ host

def _prep_inputs(inputs):
    Cs = DH ** -0.25
    f = np.float32
    bf = ml_dtypes.bfloat16
    inp = {k: np.asarray(v, dtype=f) for k, v in inputs.items()}

    def fshape(vec):
        n = vec.shape[0] // 128
        return np.ascontiguousarray(vec.reshape(n, 128).T)

    def wpack(w_t):
        # (din, dout) -> (128, KD, dout)
        dout = w_t.shape[1]
        return np.ascontiguousarray(
            w_t.reshape(KD, 128, dout).transpose(1, 0, 2))

    def cols(arr):
        """(128, ...) array of f32/bf16 -> (128, n) bf16 byte view."""
        a = np.ascontiguousarray(arr)
        a = a.reshape(128, -1)
        return a.view(bf)

    shared = {}
    shared["c_ident"] = np.eye(128, dtype=bf)

    # fold LN gamma/beta into the consumers of each LN output
    ln_fold = {"sa_wq": "1", "sa_wk": "1", "sa_wv": "1", "ca_wq": "2"}
    bias_of = {"wq": "bq", "wk": "bk", "wv": "bv"}
    biases = {p + "_" + b: inp[p + "_" + b].copy()
              for p in ("sa", "ca") for b in ("bq", "bk", "bv", "bo")}
    for pre in ("sa", "ca"):
        for nm in ("wq", "wk", "wv", "wo"):
            key = pre + "_" + nm
            w_t = np.ascontiguousarray(inp[key].T)
            if key in ln_fold:
                i = ln_fold[key]
                w_t = w_t * inp["ln%s_g" % i][:, None]
                biases[pre + "_" + bias_of[nm]] += (
                    inp[key] @ inp["ln%s_b" % i])
            shared[key] = wpack(w_t).astype(bf)
        for nm in ("bq", "bk", "bo"):
            shared[pre + "_" + nm] = fshape(biases[pre + "_" + nm])
        bvrow = np.zeros((128, D), bf)
        bvrow[0, :] = biases[pre + "_bv"].astype(bf)
        shared[pre + "_bv"] = bvrow
        wf_t = (Cs * inp[pre + "_feat"]).T          # (DH, M)
        wfc = np.zeros((128, 512), f)
        wfc[0:64, 0:256] = wf_t
        wfc[64:128, 256:512] = wf_t
        shared[pre + "_wf"] = wfc.astype(bf)
    a = np.ascontiguousarray(inp["ff_w1"].T) * inp["ln3_g"][:, None]
    b1_fold = inp["ff_b1"] + inp["ff_w1"] @ inp["ln3_b"]
    shared["w1"] = np.ascontiguousarray(
        a.reshape(KD, 128, 4, 1024).transpose(1, 2, 0, 3)).astype(bf)
    b = np.ascontiguousarray(inp["ff_w2"].T)            # (F, D)
    shared["w2"] = np.ascontiguousarray(
        b.reshape(4, KD, 128, 1024).transpose(2, 0, 1, 3)).astype(bf)
    shared["b1"] = fshape(b1_fold)
    shared["b2"] = fshape(inp["ff_b2"])

    shared_cols = {nm: cols(shared[nm]) for nm in shared}

    in_maps = []
    for core in range(8):
        b_ix, half = core // 2, core % 2
        sl = slice(half * SL, (half + 1) * SL)
        xt = np.ascontiguousarray(inp["tgt"][sl, b_ix, :].T)      # (D, SL)
        xT = np.ascontiguousarray(
            xt.reshape(KD, 128, SL).transpose(1, 0, 2))
        mt = np.ascontiguousarray(inp["memory"][sl, b_ix, :].T)
        memT = np.ascontiguousarray(
            mt.reshape(KD, 128, SL).transpose(1, 0, 2)).astype(bf)
        percore = {"xT": cols(xT), "memT": cols(memT)}
        blocks = []
        for nm, ncols in _BLOB_SPEC:
            blk = percore.get(nm)
            if blk is None:
                blk = shared_cols[nm]
            assert blk.shape == (128, ncols), (nm, blk.shape, ncols)
            blocks.append(blk)
        in_maps.append({"blob": np.concatenate(blocks, axis=1)})
    return in_maps


def _build_exec(nc, n_cores=8):
    import jax
    from jax.sharding import Mesh, PartitionSpec
    from jax.experimental.shard_map import shard_map
    from concourse import bass2jax as b2j

    b2j.install_neuronx_cc_hook()
    partition_name = (nc.partition_id_tensor.name
                      if nc.partition_id_tensor else None)
    in_names, out_names, out_avals = [], [], []
    for alloc in nc.m.functions[0].allocations:
        if not isinstance(alloc, mybir.MemoryLocationSet):
            continue
        name = alloc.memorylocations[0].name
        if alloc.kind == "ExternalInput":
            if name != partition_name:
                in_names.append(name)
        elif alloc.kind == "ExternalOutput":
            out_names.append(name)
            out_avals.append(jax.core.ShapedArray(
                tuple(alloc.tensor_shape), mybir.dt.np(alloc.dtype)))
    n_params = len(in_names)
    all_in = list(in_names) + list(out_names)
    if partition_name is not None:
        all_in.append(partition_name)

    def _body(*args):
        operands = list(args)
        if partition_name is not None:
            operands.append(b2j.partition_id_tensor())
        outs = b2j._bass_exec_p.bind(
            *operands, out_avals=tuple(out_avals), in_names=tuple(all_in),
            out_names=tuple(out_names), lowering_input_output_aliases=(),
            sim_require_finite=True, sim_require_nnan=True, nc=nc)
        return tuple(outs)

    devices = jax.devices()[:n_cores]
    mesh = Mesh(np.asarray(devices), ("core",))
    n_outs = len(out_names)
    specs = (PartitionSpec("core"),) * (n_params + n_outs)
    out_specs = (PartitionSpec("core"),) * n_outs
    donate = tuple(range(n_params, n_params + n_outs))
    sharded = jax.jit(shard_map(_body, mesh=mesh, in_specs=specs,
                                out_specs=out_specs, check_rep=False),
                      donate_argnums=donate, keep_unused=True)

    def run(in_maps, fetch=True):
        import jax as _jax
        concat = [np.concatenate([np.asarray(in_maps[c][nm])
                                  for c in range(n_cores)], axis=0)
                  for nm in in_names]
        zeros = [np.zeros((n_cores * av.shape[0], *av.shape[1:]), av.dtype)
                 for av in out_avals]
        outs = sharded(*concat, *zeros)
        if not fetch:
            _jax.block_until_ready(outs)
            return None
        return [{nm: np.asarray(outs[i]).reshape(
            n_cores, *out_avals[i].shape)[c]
            for i, nm in enumerate(out_names)} for c in range(n_cores)]

    def time_exec(in_maps, iters=8):
        """Wall-time the sharded exec with device-resident inputs."""
        import time as _time
        import jax as _jax
        from jax.sharding import NamedSharding
        sh = NamedSharding(mesh, PartitionSpec("core"))
        concat = [np.concatenate([np.asarray(in_maps[c][nm])
                                  for c in range(n_cores)], axis=0)
                  for nm in in_names]
        dev_in = _jax.device_put(concat, [sh] * len(concat))
        _jax.block_until_ready(dev_in)
        zeros = [np.zeros((n_cores * av.shape[0], *av.shape[1:]), av.dtype)
                 for av in out_avals]
        times = []
        for _ in range(iters):
            zd = _jax.device_put(zeros, [sh] * len(zeros))
            _jax.block_until_ready(zd)
            t0 = _time.time()
            outs = sharded(*dev_in, *zd)
            _jax.block_until_ready(outs)
            times.append(_time.time() - t0)
        return times

    run.in_names = in_names
    run.time_exec = time_exec
    run.sharded = sharded
    run.mesh = mesh
    run.out_avals = out_avals
    run.n_params = n_params
    return run


def _get_exec():
    if "exec" not in _CACHE:
        nc = build_nc()
        _CACHE["exec"] = _build_exec(nc)
    return _CACHE["exec"]


def kernel(**inputs):
    run = _get_exec()
    in_maps = _prep_inputs(inputs)
    res = run(in_maps)
    out = np.empty((S, B, D), np.float32)
    for c in range(8):
        b_ix, half = c // 2, c % 2
        slab = res[c]["outT"]                       # (128, KD, SL)
        out[half * SL:(half + 1) * SL, b_ix, :] = (
            slab.transpose(1, 0, 2).reshape(D, SL).T)
    return out


# revision 30
# speedup vs baseline: 1.0819x; 1.0600x over previous
"""Trainium2 Bass kernel for nn_CustomDecoderLayer (FAVOR+ decoder layer).

Sharding: 8 cores = 4 batches x 2 sequence halves (S'=1024 tokens/core),
full D/heads/F per core. The only collectives are one bf16 AllReduce-add
of the FAVOR+ kv summary (65 x H*258, ~0.5 MB) per attention block,
within each core pair; column 257 carries exp(8*(local_stab-8)) so the
same AllReduce transports the k-stabilizer (recovered via x^(-1/8)).
Both ARs are launched early and covered by independent compute (the
cross-attn k/v path does not depend on self-attn output; its projection
units run as fillers inside the self-attn phi(k) pipeline). The
residual stays in SBUF for the whole kernel; LayerNorm gamma/beta are
folded into the downstream projection weights host-side.

All inputs ride in ONE host-packed bf16 blob per core (f32 slices are
bitcast views) -- per-exec dispatch overhead scales with the input
buffer count (~20us/buffer), so 30 tensors -> 1 saves ~0.6 ms/exec.
"""
import sys
sys.path.insert(0, "/opt/trn_rl_repo")
from contextlib import ExitStack

import numpy as np
import ml_dtypes

import concourse.bass as bass
import concourse.mybir as mybir
import concourse.tile as tile
from concourse import bacc, bass_isa

f32 = mybir.dt.float32
f32r = mybir.dt.float32r
bf16 = mybir.dt.bfloat16
AF = mybir.ActivationFunctionType
AX = mybir.AxisListType
ALU = mybir.AluOpType

D, H, DH, M = 1024, 16, 64, 256
S, B, F = 2048, 4, 4096
SL = 1024                     # tokens per core (one seq half)
KD = D // 128                 # 8
RT = SL // 128                # 8
NCH = SL // 512               # 2
NP = H // 2                   # 8 head pairs
C2 = 0.5 * (DH ** -0.5)       # 0.0625, exact in bf16
EPS16 = 1.0e-6 * 16.0
RG = [[0, 1], [2, 3], [4, 5], [6, 7]]

_CACHE = {}

# ---------------------------------------------------------------- blob
# One bf16 dram tensor per core; (name, bf16-cols, note). f32 payloads
# occupy 2 cols per value and are bitcast device-side.

_BLOB_SPEC = [
    ("xT", 2 * KD * SL),        # f32 (128, KD, SL)
    ("sa_wk", KD * D),          # bf16 (128, KD, D)
    ("sa_bk", 2 * KD),          # f32 (128, KD)
    ("sa_wf", 512),             # bf16 (128, 512)
    ("sa_wv", KD * D),
    ("sa_bv", D),               # bf16 (1, D) in partition 0
    ("memT", KD * SL),          # bf16 (128, KD, SL)
    ("ca_wk", KD * D),
    ("ca_bk", 2 * KD),
    ("ca_wf", 512),
    ("ca_wv", KD * D),
    ("ca_bv", D),
    ("sa_wq", KD * D),
    ("sa_bq", 2 * KD),
    ("c_ident", 128),           # bf16 (128, 128)
    ("sa_wo", KD * D),
    ("sa_bo", 2 * KD),
    ("ca_wq", KD * D),
    ("ca_bq", 2 * KD),
    ("ca_wo", KD * D),
    ("ca_bo", 2 * KD),
    ("w1", 4 * KD * 1024),      # bf16 (128, 4, KD, 1024)
    ("b1", 2 * 32),             # f32 (128, 32)
    ("w2", 4 * KD * 1024),
    ("b2", 2 * KD),             # f32 (128, KD)
]
_BLOB_OFF = {}
_off = 0
for _nm, _c in _BLOB_SPEC:
    _BLOB_OFF[_nm] = _off
    _off += _c
BLOB_COLS = _off


def _bslice(blob, name):
    off = _BLOB_OFF[name]
    cols = dict(_BLOB_SPEC)[name]
    return blob[:, off:off + cols]


def _ln(nc, tc, ctx, x_t, out_t, c_invd, c_eps):
    """LayerNorm F-layout: x_t (128, KD, SL) f32r -> out_t bf16.

    Stats for both chunks first, then normalize kd-outer so consumers
    that read t2 per-kd (full SL) unblock as early as possible. The
    affine gamma/beta are folded into downstream weights host-side, so
    this emits plain (x - mu) * rstd."""
    ps = ctx.enter_context(tc.tile_pool(name="lnps", bufs=2, space="PSUM"))
    sb = ctx.enter_context(tc.tile_pool(name="lnsb", bufs=3))
    mus, rstds = [], []
    for ch in range(NCH):
        cs = bass.ts(ch, 512)
        mv = ps.tile([128, 2, 512], f32, tag="ln_ps", name="ln_ps")
        for kd in range(KD):
            x2 = sb.tile([128, 512], f32r, tag="ln_x2", name="ln_x2")
            nc.scalar.activation(x2[:], x_t[:, kd, cs].bitcast(f32),
                                 AF.Square)
            nc.tensor.matmul(mv[:, 0, :], c_invd[:], x_t[:, kd, cs],
                             start=(kd == 0), stop=(kd == KD - 1),
                             skip_group_check=True)
            nc.tensor.matmul(mv[:, 1, :], c_invd[:], x2[:],
                             start=(kd == 0), stop=(kd == KD - 1),
                             skip_group_check=True)
        mu = sb.tile([128, 512], f32, tag="ln_mu", name="ln_mu", bufs=2)
        nc.any.tensor_copy(mu[:], mv[:, 0, :])
        mu2 = sb.tile([128, 512], f32, tag="ln_mu2", name="ln_mu2")
        nc.vector.tensor_mul(mu2[:], mu[:], mu[:])
        var = sb.tile([128, 512], f32, tag="ln_var", name="ln_var")
        nc.vector.tensor_sub(var[:], mv[:, 1, :], mu2[:])
        sd = sb.tile([128, 512], f32, tag="ln_sd", name="ln_sd")
        nc.scalar.activation(sd[:], var[:], AF.Sqrt, bias=c_eps[:])
        rstd = sb.tile([128, 512], f32, tag="ln_rstd", name="ln_rstd")
        nc.vector.reciprocal(rstd[:], sd[:])
        mus.append(mu)
        rstds.append(rstd)
    for kd in range(KD):
        for ch in range(NCH):
            cs = bass.ts(ch, 512)
            # split normalize units across DVE and Pool (~10:6)
            eng = nc.vector if (kd * NCH + ch) % 8 < 5 else nc.gpsimd
            xm = sb.tile([128, 512], f32, tag="ln_xm", name="ln_xm")
            eng.tensor_sub(xm[:], x_t[:, kd, cs], mus[ch][:])
            eng.tensor_mul(out_t[:, kd, cs], xm[:], rstds[ch][:])


def _load_w(nc, wp, w_ap):
    """One-DMA load of a host-packed (128, KD, 1024) bf16 weight."""
    w_sb = wp.tile([128, KD, D], bf16, tag="w", name="w_sb")
    nc.sync.dma_start(out=w_sb[:], in_=w_ap)
    return w_sb


def _proj_F_unit(nc, ps, src_t, w_sb, b_t, out_t, m, dve_evac=False):
    """One m-tile of an F-layout projection."""
    o_ps = ps.tile([128, SL], f32, tag="proj_ps", name="proj_ps")
    for kd in range(KD):
        for ch in range(NCH):
            cs = bass.ts(ch, 512)
            nc.tensor.matmul(o_ps[:, cs], w_sb[:, kd, m * 128:(m + 1) * 128],
                             src_t[:, kd, cs],
                             start=(kd == 0), stop=(kd == KD - 1),
                             skip_group_check=True)
    nc.any.tensor_scalar_add(out_t[:, m, :], o_ps[:], b_t[:, m:m + 1])


def _proj_F(nc, tc, ctx, src_t, w_sb, b_t, out_t):
    """F-layout projection: out[dout, tok]. src (128, KD, SL) bf16,
    w_sb (128, KD, D) [din, dout]. Full-SL moving operand (bf16)."""
    ps = ctx.enter_context(tc.tile_pool(name="pfps", bufs=2, space="PSUM"))
    for m in range(KD):
        _proj_F_unit(nc, ps, src_t, w_sb, b_t, out_t, m)


def _proj_R_unit(nc, ps, src_t, w_sb, bv_rep, v_aug, rt):
    """One token-tile of the R-layout V projection."""
    rs = bass.ts(rt, 128)
    v_ps = ps.tile([128, SL], f32, tag="v_ps", name="v_ps")
    for kd in range(KD):
        for ch in range(NCH):
            cs = bass.ts(ch, 512)
            nc.tensor.matmul(v_ps[:, cs], src_t[:, kd, rs],
                             w_sb[:, kd, cs],
                             start=(kd == 0), stop=(kd == KD - 1),
                             skip_group_check=True)
    nc.any.tensor_add(v_aug[:, rt, :, 0:64], v_ps[:], bv_rep[:])


def _proj_R_vaug(nc, tc, ctx, src_t, w_sb, bv_rep, v_aug):
    """R-layout V projection into v_aug (128, RT, H, 65) bf16."""
    ps = ctx.enter_context(tc.tile_pool(name="pvps", bufs=2, space="PSUM"))
    for rt in range(RT):
        _proj_R_unit(nc, ps, src_t, w_sb, bv_rep, v_aug, rt)


def _phi_k_kv(nc, tc, ctx, kT, v_aug, wf_cat, c_negblk, c_b64, kv_sb,
              fillers=(), half_cb=None, ebufs=2, kvbufs=1, pjbufs=2):
    """phi(k) + local kv partials, all heads. E carries no stabilizer;
    row 65 of kv_sb gets exp(8*(local_stab - 8)) so the single
    AllReduce-add also transports the stab: the consume side recovers
    ~exp(-max_stab) as (sum)^(-1/8) * e^-8 (error <= log2/8 in the
    exponent, which only perturbs the eps weighting by <1%).

    `fillers` are independent PE-dense work units (closures) interleaved
    between pipeline stages; each pair's kv matmuls are emitted one pair
    late so the in-order PE queue never waits on the Act-engine exps."""
    ps_pj = ctx.enter_context(tc.tile_pool(name="pkpj", bufs=pjbufs,
                                           space="PSUM"))
    ps_ns = ctx.enter_context(tc.tile_pool(name="pkns", bufs=1, space="PSUM"))
    ps_kv = ctx.enter_context(tc.tile_pool(name="pkkv", bufs=kvbufs,
                                           space="PSUM"))
    sb = ctx.enter_context(tc.tile_pool(name="pksb", bufs=3))
    ep = ctx.enter_context(tc.tile_pool(name="pkep", bufs=ebufs))
    nc.vector.memset(kv_sb[:, :, 257:258], 0.0)
    fill_iter = iter(fillers)

    def emit_fill(n):
        for _ in range(n):
            f = next(fill_iter, None)
            if f is not None:
                f()

    def stage_kv(g, E_t):
        for h in range(2):
            kv_ps = ps_kv.tile([65, 257], f32, tag="kv_ps", name="kv_ps")
            for rt in range(RT):
                nc.tensor.matmul(kv_ps[:], v_aug[:, rt, 2 * g + h, :],
                                 E_t[:, rt, h, :], start=(rt == 0),
                                 stop=(rt == RT - 1), skip_group_check=True)
            nc.any.tensor_copy(kv_sb[0:65, 2 * g + h, 0:257], kv_ps[:])

    prev = None
    for g in range(NP):
        E_t = ep.tile([128, RT, 2, 257], bf16, tag="E_t", name="E_t")
        nc.vector.memset(E_t[:, :, :, 256:257], EPS16)
        nsq_all = sb.tile([128, RT, 2], f32, tag="nsq_all", name="nsq_all")
        for rt in range(RT):
            rs = bass.ts(rt, 128)
            k2 = sb.tile([128, 128], bf16, tag="k2", name="k2")
            nc.vector.tensor_mul(k2[:], kT[:, g, rs], kT[:, g, rs])
            pj = ps_pj.tile([128, 2, 256], f32, tag="pj", name="pj_k")
            nc.tensor.matmul(pj[:, :, :], kT[:, g, rs], wf_cat[:])
            nsq = ps_ns.tile([128, 2], f32, tag="nsq", name="nsq_k")
            nc.tensor.matmul(nsq[:], k2[:], c_negblk[:])
            nc.any.tensor_copy(nsq_all[:, rt, :], nsq[:])
            for h in range(2):
                nc.scalar.activation(E_t[:, rt, h, 0:256], pj[:, h, :],
                                     AF.Exp, bias=nsq_all[:, rt, h:h + 1])
        # stabilizer recovered post-exp: max_m E = e^(stab_tok - sq), so
        # stab_tok = ln(max_m E) + sq; one wide bf16 reduce replaces 8
        # PSUM f32 reduce_max ops.
        # transport value exp(8*(stab-8)) = maxE^8 * e^(8*sq-64), computed
        # without Ln (avoids Act table reloads between the Exps): maxE =
        # e^(stab_tok - sq), cube-squared on DVE; the e^(8*sq-64) factor is
        # an Exp (same Act table as phi).
        maxE = sb.tile([128, RT, 2], f32, tag="maxE", name="maxE")
        nc.vector.reduce_max(maxE[:], E_t[:, :, :, 0:256], axis=AX.X)
        m2 = sb.tile([128, RT, 2], f32, tag="m2", name="m2")
        nc.vector.tensor_mul(m2[:], maxE[:], maxE[:])
        nc.vector.tensor_mul(m2[:], m2[:], m2[:])
        nc.vector.tensor_mul(m2[:], m2[:], m2[:])
        e8 = sb.tile([128, RT, 2], f32, tag="e8", name="e8")
        nc.scalar.activation(e8[:], nsq_all[:], AF.Exp, scale=-8.0,
                             bias=c_b64[:])
        tv = sb.tile([128, RT, 2], f32, tag="tv", name="tv")
        nc.vector.tensor_mul(tv[:], m2[:], e8[:])
        stab_run = sb.tile([128, 2], f32, tag="stab_run", name="stab_run")
        nc.vector.reduce_max(stab_run[:],
                             tv[:].rearrange("p r h -> p h r"),
                             axis=AX.X)
        stab_rep = sb.tile([128, 2], f32, tag="stab_rep", name="stab_rep")
        nc.gpsimd.partition_all_reduce(stab_rep[:], stab_run[:], channels=128,
                                       reduce_op=bass_isa.ReduceOp.max)
        nc.any.tensor_copy(kv_sb[0:1, 2 * g:2 * g + 2, 257:258],
                           stab_rep[0:1, :])
        emit_fill(2)
        if prev is not None:
            stage_kv(*prev)
            if prev[0] == 3 and half_cb is not None:
                half_cb(0)
        prev = (g, E_t)
    stage_kv(*prev)
    if half_cb is not None:
        half_cb(1)
    emit_fill(len(fillers))


def _kv_consume(nc, tc, ctx, kv_out, ident_bf, kvT, kvcolT, h0=0, nh=H):
    """Load AR result; recover s = ~exp(-stab_max) from the summed
    exp(8*(stab-8)) row via three chained sqrts + fast reciprocal;
    scale kv by s and fold in the (unscaled) eps column; transpose
    -> kvT. Also emits kvcolT[1, h, 65] = sum_m kva (the rank-1
    stationary for the exact q-side eps correction)."""
    sb = ctx.enter_context(tc.tile_pool(name="kcsb", bufs=2))
    kvp = ctx.enter_context(tc.tile_pool(name="kckv", bufs=1))
    ps_tp = ctx.enter_context(tc.tile_pool(name="kctp", bufs=2, space="PSUM"))
    kv2 = kvp.tile([65, nh, 258], bf16, name="kv2")
    nc.sync.dma_start(out=kv2[:], in_=kv_out[:])
    s_row = sb.tile([1, nh], f32, tag="s_row", name="s_row")
    nc.any.tensor_copy(s_row[:], kv2[0:1, :, 257:258])
    r1 = sb.tile([1, nh], f32, tag="r1", name="r1")
    nc.scalar.activation(r1[:], s_row[:], AF.Sqrt)
    r2 = sb.tile([1, nh], f32, tag="r2", name="r2")
    nc.scalar.activation(r2[:], r1[:], AF.Sqrt)
    r3 = sb.tile([1, nh], f32, tag="r3", name="r3")
    nc.scalar.activation(r3[:], r2[:], AF.Sqrt)
    r4 = sb.tile([1, nh], f32, tag="r4", name="r4")
    nc.vector.reciprocal(r4[:], r3[:])
    s_t = sb.tile([1, nh], f32, tag="s_t", name="s_t")
    nc.vector.tensor_scalar_mul(s_t[:], r4[:], float(np.exp(-8.0)))
    s_bc = sb.tile([128, nh], f32, tag="s_bc", name="s_bc")
    nc.gpsimd.partition_broadcast(s_bc[:], s_t[:], channels=128)
    for h in range(nh):
        csum = sb.tile([65, 1], f32, tag="csum", name="csum")
        nc.any.tensor_copy(csum[:], kv2[0:65, h, 256:257])
        kva = sb.tile([65, 256], bf16, tag="kva", name="kva")
        nc.vector.tensor_scalar(kva[:], kv2[0:65, h, 0:256],
                                s_bc[0:65, h:h + 1], csum[:],
                                ALU.mult, ALU.add)
        kvcol = sb.tile([65, 1], f32, tag="kvcol", name="kvcol")
        nc.vector.reduce_sum(kvcol[:], kva[:], axis=AX.X)
        kvcolb = sb.tile([65, 1], bf16, tag="kvcolb", name="kvcolb")
        nc.any.tensor_copy(kvcolb[:], kvcol[:])
        tpc = ps_tp.tile([1, 65], bf16, tag="tp_kv", name="tp_kvc")
        nc.tensor.transpose(tpc[:], kvcolb[:], ident_bf[0:65, 0:65])
        nc.any.tensor_copy(kvcolT[0:1, h0 + h, :], tpc[:])
        for mt in range(2):
            tp = ps_tp.tile([128, 65], bf16, tag="tp_kv", name="tp_kv")
            nc.tensor.transpose(tp[:], kva[0:65, mt * 128:(mt + 1) * 128],
                                ident_bf[0:65, 0:65])
            nc.any.tensor_copy(kvT[:, h0 + h, mt, :], tp[:])


def _phi_q_out(nc, tc, ctx, qT, kvT, kvcolT, wf_cat, c_negblk, ident_bf,
               c_lneps, attn_t, pairs=range(NP)):
    """phi(q), exact reference semantics, stabilizer-free.

    A per-token scale on pq cancels exactly in out/z, so pq is used
    UNSCALED: pq_u = e^(proj - sq) (bf16-safe, <= e^5.3 here). The only
    place the reference stabilizer matters is the relative weight of
    its +eps term, which equals a rank-1 correction
    eps16 * e^(stab_tok) * colsum_m(kva) -- added exactly via a K=1
    matmul accumulated into the same PSUM group (e^(stab_tok) =
    rowmax(pq_u) * e^(sq), both cheap post-exp byproducts). This
    removes all per-rt PSUM reductions and the scale barrier from the
    q path."""
    ps_pj = ctx.enter_context(tc.tile_pool(name="pqpj", bufs=2, space="PSUM"))
    ps_ns = ctx.enter_context(tc.tile_pool(name="pqns", bufs=1, space="PSUM"))
    ps_tp = ctx.enter_context(tc.tile_pool(name="pqtp", bufs=3, space="PSUM"))
    ps_o = ctx.enter_context(tc.tile_pool(name="pqo", bufs=2, space="PSUM"))
    sb = ctx.enter_context(tc.tile_pool(name="pqsb", bufs=3))
    pqrp = ctx.enter_context(tc.tile_pool(name="pqrp", bufs=2))
    pqp = ctx.enter_context(tc.tile_pool(name="pqpq", bufs=2))
    for g in pairs:
        pqR = pqrp.tile([128, RT, 2, 256], bf16, tag="pqR", name="pqR")
        nsq_all = sb.tile([128, RT, 2], f32, tag="nsq_all", name="nsq_all")
        rmax = sb.tile([128, RT, 2], f32, tag="rmax_q", name="rmax_q")
        enq = sb.tile([128, RT, 2], f32, tag="enq", name="enq")
        eff = sb.tile([128, RT, 2], bf16, tag="eff", name="eff")
        effT = sb.tile([1, 2, SL], bf16, tag="effT", name="effT", bufs=2)
        pqT = pqp.tile([128, 2, 2, SL], bf16, tag="pqT", name="pqT")
        for rt in range(RT):
            rs = bass.ts(rt, 128)
            q2 = sb.tile([128, 128], bf16, tag="q2", name="q2")
            nc.vector.tensor_mul(q2[:], qT[:, g, rs], qT[:, g, rs])
            pj = ps_pj.tile([128, 2, 256], f32, tag="pj", name="pj_q")
            nc.tensor.matmul(pj[:, :, :], qT[:, g, rs], wf_cat[:])
            nsq = ps_ns.tile([128, 2], f32, tag="nsq", name="nsq_q")
            nc.tensor.matmul(nsq[:], q2[:], c_negblk[:])
            nc.any.tensor_copy(nsq_all[:, rt, :], nsq[:])
            nc.scalar.activation(pqR[:, rt, :, :], pj[:, :, :], AF.Exp)
            tp = ps_tp.tile([128, 2, 2, 128], bf16, tag="tp_pq", name="tp_pq")
            for h in range(2):
                for mt in range(2):
                    nc.tensor.transpose(
                        tp[:, h, mt, :],
                        pqR[:, rt, h, mt * 128:(mt + 1) * 128], ident_bf[:])
            nc.any.tensor_copy(pqT[:, :, :, rs], tp[:])
            if rt % 4 == 3:
                # rank-1 eps factors for this 512-token half:
                # eff = eps16 * e^(stab_tok) = rowmax(pq_u) * eps16*e^(sq)
                hh = slice(rt - 3, rt + 1)
                nc.vector.reduce_max(rmax[:, hh, :], pqR[:, hh, :, :],
                                     axis=AX.X)
                nc.scalar.activation(enq[:, hh, :], nsq_all[:, hh, :],
                                     AF.Exp, scale=-1.0, bias=c_lneps[:])
                nc.vector.tensor_mul(eff[:, hh, :], rmax[:, hh, :],
                                     enq[:, hh, :])
                for rr in range(rt - 3, rt + 1):
                    tpe = ps_tp.tile([1, 2, 128], bf16, tag="tp_pq",
                                     name="tp_eff")
                    for h in range(2):
                        nc.tensor.transpose(tpe[:, h, :],
                                            eff[:, rr, h:h + 1],
                                            ident_bf[:])
                    nc.any.tensor_copy(effT[:, :, bass.ts(rr, 128)],
                                       tpe[:])
        for h in range(2):
            hp = slice(64 * h, 64 * h + 64)
            for ch in range(NCH):
                cs = bass.ts(ch, 512)
                o_ps = ps_o.tile([65, 512], f32, tag="o_ps", name="o_ps")
                for mt in range(2):
                    nc.tensor.matmul(o_ps[:], kvT[:, 2 * g + h, mt, :],
                                     pqT[:, h, mt, cs], start=(mt == 0),
                                     stop=False, skip_group_check=True)
                nc.tensor.matmul(o_ps[:], kvcolT[0:1, 2 * g + h, :],
                                 effT[0:1, h, cs], start=False, stop=True,
                                 skip_group_check=True)
                zr = sb.tile([1, 512], f32, tag="zr", name="zr")
                nc.vector.reciprocal(zr[:], o_ps[64:65, :])
                zb = sb.tile([64, 512], f32, tag="zb", name="zb", bufs=2)
                nc.gpsimd.partition_broadcast(zb[:], zr[:], channels=64)
                nc.any.tensor_mul(attn_t[hp, g, cs], o_ps[0:64, :],
                                  zb[:])


def _proj_add(nc, tc, ctx, src_t, w_sb, b_t, x_t):
    """Wo-style projection (bf16 src); adds result into x_t (f32r)."""
    ps = ctx.enter_context(tc.tile_pool(name="waps", bufs=2, space="PSUM"))
    for m in range(KD):
        o_ps = ps.tile([128, SL], f32, tag="wa_ps", name="wa_ps")
        for kd in range(KD):
            for ch in range(NCH):
                cs = bass.ts(ch, 512)
                nc.tensor.matmul(o_ps[:, cs],
                                 w_sb[:, kd, m * 128:(m + 1) * 128],
                                 src_t[:, kd, cs],
                                 start=(kd == 0), stop=(kd == KD - 1),
                                 skip_group_check=True)
        nc.vector.scalar_tensor_tensor(
            x_t[:, m, :], o_ps[:], b_t[:, m:m + 1], x_t[:, m, :],
            ALU.add, ALU.add)


def build_nc(no_ar=False):
    nc = bacc.Bacc("TRN2", target_bir_lowering=False, debug=False,
                   num_devices=8)

    blob = nc.dram_tensor("blob", [128, BLOB_COLS], bf16,
                          kind="ExternalInput").ap()

    def wslice(name):
        return _bslice(blob, name).rearrange("p (k d) -> p k d", k=KD)

    def fslice(name, n):
        return _bslice(blob, name).bitcast(f32)

    outT = nc.dram_tensor("outT", [128, KD, SL], f32,
                          kind="ExternalOutput").ap()

    with tile.TileContext(nc) as tc:
        with ExitStack() as top:
            dram = top.enter_context(tc.tile_pool(name="dram", bufs=1,
                                                  space="DRAM"))
            ccs = {}
            for half in "ab":
                ccs["sa_kv_in_" + half] = dram.tile(
                    [65, 8 * 258], bf16, name="sa_kv_in_" + half)
                ccs["sa_kv_out_" + half] = dram.tile(
                    [65, 8 * 258], bf16, name="sa_kv_out_" + half)
            ccs["ca_kv_in"] = dram.tile([65, H * 258], bf16,
                                        name="ca_kv_in")
            ccs["ca_kv_out"] = dram.tile([65, H * 258], bf16,
                                         name="ca_kv_out")

            # persistent activations first so their DMAs lead the queue
            const = top.enter_context(tc.tile_pool(name="const", bufs=1))
            xp = top.enter_context(tc.tile_pool(name="xp", bufs=1))
            x_t = xp.tile([128, KD, SL], f32r, name="x_t")
            xT = _bslice(blob, "xT").bitcast(f32r).rearrange(
                "p (k s) -> p k s", k=KD)
            for kd in range(KD):
                nc.sync.dma_start(out=x_t[:, kd, :], in_=xT[:, kd, :])

            wp = top.enter_context(tc.tile_pool(name="wp", bufs=2))
            # SA front weights ride right behind x so the first
            # projection is never DMA-starved; memory comes after.
            w_k = _load_w(nc, wp, wslice("sa_wk"))
            cb = {}
            for pre in ("sa", "ca"):
                for nm in ("bq", "bk", "bo"):
                    key = pre + "_" + nm
                    t = const.tile([128, KD], f32, name=pre + nm)
                    if key == "sa_bk":
                        nc.sync.dma_start(out=t[:], in_=fslice(key, KD))
                    cb[key] = t
                t = const.tile([1, D], bf16, name=pre + "bv")
                cb[pre + "_bv"] = t
                wfc = const.tile([128, 512], bf16, name=pre + "wfc")
                cb[pre + "_wf"] = wfc
            nc.sync.dma_start(out=cb["sa_wf"][:], in_=_bslice(blob, "sa_wf"))
            w_v = _load_w(nc, wp, wslice("sa_wv"))
            nc.sync.dma_start(out=cb["sa_bv"][:],
                              in_=_bslice(blob, "sa_bv")[0:1, :])
            memstack = ExitStack()
            memp = memstack.enter_context(tc.tile_pool(name="memp", bufs=1,
                                                       side="right"))
            mem_t = memp.tile([128, KD, SL], bf16, name="mem_t")
            nc.sync.dma_start(out=mem_t[:],
                              in_=_bslice(blob, "memT").rearrange(
                                  "p (k s) -> p k s", k=KD))
            for key in ("sa_bq", "sa_bo", "ca_bq", "ca_bk", "ca_bo"):
                nc.sync.dma_start(out=cb[key][:], in_=fslice(key, KD))
            nc.sync.dma_start(out=cb["ca_wf"][:], in_=_bslice(blob, "ca_wf"))
            nc.sync.dma_start(out=cb["ca_bv"][:],
                              in_=_bslice(blob, "ca_bv")[0:1, :])

            # on-device constants (no DMA)
            c_invd = const.tile([128, 128], f32, name="c_invd")
            nc.vector.memset(c_invd[:], 1.0 / D)
            c_invd_r = c_invd[:].bitcast(f32r)
            c_negblk = const.tile([128, 2], bf16, name="c_negblk")
            nc.vector.memset(c_negblk[:], 0.0)
            nc.vector.memset(c_negblk[0:64, 0:1], -C2)
            nc.vector.memset(c_negblk[64:128, 1:2], -C2)
            ident_bf = const.tile([128, 128], bf16, name="ident_bf")
            nc.sync.dma_start(out=ident_bf[:], in_=_bslice(blob, "c_ident"))
            c_eps = const.tile([128, 1], f32, name="c_eps")
            nc.vector.memset(c_eps[:], 1.0e-5)
            c_lneps = const.tile([128, 1], f32, name="c_lneps")
            nc.vector.memset(c_lneps[:], float(np.log(EPS16)))
            c_b64 = const.tile([128, 1], f32, name="c_b64")
            nc.vector.memset(c_b64[:], -64.0)
            for nm, shp in (("b1", [128, 32]), ("b2", [128, KD])):
                t = const.tile(shp, f32, name=nm)
                nc.sync.dma_start(out=t[:], in_=fslice(nm, shp[1]))
                cb[nm] = t

            t2_t = xp.tile([128, KD, SL], bf16, name="t2_t")

            def launch_ar(pre, kv_sb):
                nc.sync.dma_start(out=ccs[pre + "_kv_in"][:], in_=kv_sb[:])
                if no_ar:
                    nc.sync.dma_start(out=ccs[pre + "_kv_out"][:],
                                      in_=ccs[pre + "_kv_in"][:])
                else:
                    nc.gpsimd.collective_compute(
                        "AllReduce", ALU.add, replica_groups=RG,
                        ins=[ccs[pre + "_kv_in"].opt()],
                        outs=[ccs[pre + "_kv_out"].opt()])

            # ---------- LN1 + SA front
            with ExitStack() as ph:
                _ln(nc, tc, ph, x_t, t2_t, c_invd_r, c_eps)
            # ---------- SA front (k/v proj + phi), CA proj interleaved
            safr = ExitStack()
            fr = safr.enter_context(tc.tile_pool(name="safr", bufs=1))
            kT_sa = fr.tile([128, KD, SL], bf16, name="sa_kT")
            with ExitStack() as ph:
                _proj_F(nc, tc, ph, t2_t, w_k, cb["sa_bk"], kT_sa)
            vaug_sa = fr.tile([128, RT, H, 65], bf16, name="sa_vaug")
            nc.vector.memset(vaug_sa[:, :, :, 64:65], 1.0)
            bvr_sa = fr.tile([128, D], bf16, name="sa_bvr")
            nc.gpsimd.partition_broadcast(bvr_sa[:], cb["sa_bv"][:],
                                          channels=128)
            with ExitStack() as ph:
                _proj_R_vaug(nc, tc, ph, t2_t, w_v, bvr_sa, vaug_sa)
            kvsb_sa = fr.tile([65, H, 258], bf16, name="sa_kvsb")

            # CA front tiles + weights prepped now; its projection units
            # run as fillers inside the SA phi pipeline
            cafr = ExitStack()
            cfr = cafr.enter_context(tc.tile_pool(name="cafr", bufs=1,
                                                  side="right"))
            ca_ps_stack = ExitStack()
            ca_ps_k = ca_ps_stack.enter_context(
                tc.tile_pool(name="capfps", bufs=1, space="PSUM"))
            ca_ps_v = ca_ps_stack.enter_context(
                tc.tile_pool(name="capvps", bufs=1, space="PSUM"))
            w_k_ca = _load_w(nc, wp, wslice("ca_wk"))
            w_v_ca = _load_w(nc, wp, wslice("ca_wv"))
            kT_ca = cfr.tile([128, KD, SL], bf16, name="ca_kT")
            vaug_ca = cfr.tile([128, RT, H, 65], bf16, name="ca_vaug")
            nc.vector.memset(vaug_ca[:, :, :, 64:65], 1.0)
            bvr_ca = cfr.tile([128, D], bf16, name="ca_bvr")
            nc.gpsimd.partition_broadcast(bvr_ca[:], cb["ca_bv"][:],
                                          channels=128)
            kvsb_ca = cfr.tile([65, H, 258], bf16, name="ca_kvsb")
            ca_units = (
                [lambda m=m: _proj_F_unit(nc, ca_ps_k, mem_t, w_k_ca,
                                          cb["ca_bk"], kT_ca, m,
                                          dve_evac=True)
                 for m in range(KD)] +
                [lambda rt=rt: _proj_R_unit(nc, ca_ps_v, mem_t, w_v_ca,
                                            bvr_ca, vaug_ca, rt)
                 for rt in range(RT)])

            def sa_half_ar(i):
                half = "ab"[i]
                hs = slice(8 * i, 8 * i + 8)
                nc.sync.dma_start(out=ccs["sa_kv_in_" + half][:],
                                  in_=kvsb_sa[0:65, hs, :])
                if no_ar:
                    nc.sync.dma_start(out=ccs["sa_kv_out_" + half][:],
                                      in_=ccs["sa_kv_in_" + half][:])
                else:
                    nc.gpsimd.collective_compute(
                        "AllReduce", ALU.add, replica_groups=RG,
                        ins=[ccs["sa_kv_in_" + half].opt()],
                        outs=[ccs["sa_kv_out_" + half].opt()])

            with ExitStack() as ph:
                _phi_k_kv(nc, tc, ph, kT_sa, vaug_sa, cb["sa_wf"],
                          c_negblk, c_b64, kvsb_sa, fillers=ca_units,
                          half_cb=sa_half_ar)
            safr.close()
            ca_ps_stack.close()

            # ---------- CA phi; SA q-proj interleaved (covers SA AR)
            qsap = top.enter_context(tc.tile_pool(name="qsap", bufs=1))
            qT_sa = qsap.tile([128, KD, SL], bf16, name="qT_sa")
            qps = cafr.enter_context(
                tc.tile_pool(name="qpfps", bufs=1, space="PSUM"))
            w_q = _load_w(nc, wp, wslice("sa_wq"))
            q_units = [lambda m=m: _proj_F_unit(nc, qps, t2_t, w_q,
                                                cb["sa_bq"], qT_sa, m,
                                                dve_evac=True)
                       for m in range(KD)]
            with ExitStack() as ph:
                _phi_k_kv(nc, tc, ph, kT_ca, vaug_ca, cb["ca_wf"],
                          c_negblk, c_b64, kvsb_ca, fillers=q_units,
                          kvbufs=2, pjbufs=3)
            launch_ar("ca", kvsb_ca)
            cafr.close()
            memstack.close()

            # ---------- SA back
            sabk = ExitStack()
            bk = sabk.enter_context(tc.tile_pool(name="sabk", bufs=1))
            w_o = _load_w(nc, wp, wslice("sa_wo"))
            kvT_sa = bk.tile([128, H, 2, 65], bf16, name="sa_kvT")
            kvcolT_sa = bk.tile([1, H, 65], bf16, name="sa_kvcolT")
            attn_sa = bk.tile([128, KD, SL], bf16, name="sa_attn")
            for i in range(2):
                with ExitStack() as ph:
                    _kv_consume(nc, tc, ph, ccs["sa_kv_out_" + "ab"[i]],
                                ident_bf, kvT_sa, kvcolT_sa, h0=8 * i, nh=8)
                with ExitStack() as ph:
                    _phi_q_out(nc, tc, ph, qT_sa, kvT_sa, kvcolT_sa,
                               cb["sa_wf"], c_negblk, ident_bf, c_lneps,
                               attn_sa, pairs=range(4 * i, 4 * i + 4))
            with ExitStack() as ph:
                _proj_add(nc, tc, ph, attn_sa, w_o, cb["sa_bo"], x_t)
            sabk.close()

            # ---------- CA back: consume AR early, then LN2 + q proj
            cabk = ExitStack()
            cbk = cabk.enter_context(tc.tile_pool(name="cabk", bufs=1))
            kvT_ca = cbk.tile([128, H, 2, 65], bf16, name="ca_kvT")
            kvcolT_ca = cbk.tile([1, H, 65], bf16, name="ca_kvcolT")
            with ExitStack() as ph:
                _kv_consume(nc, tc, ph, ccs["ca_kv_out"], ident_bf, kvT_ca,
                            kvcolT_ca)
            with ExitStack() as ph:
                _ln(nc, tc, ph, x_t, t2_t, c_invd_r, c_eps)
            w_q2 = _load_w(nc, wp, wslice("ca_wq"))
            qT_ca = cbk.tile([128, KD, SL], bf16, name="qT_ca")
            with ExitStack() as ph:
                _proj_F(nc, tc, ph, t2_t, w_q2, cb["ca_bq"], qT_ca)
            w_o2 = _load_w(nc, wp, wslice("ca_wo"))
            attn_ca = cbk.tile([128, KD, SL], bf16, name="ca_attn")
            with ExitStack() as ph:
                _phi_q_out(nc, tc, ph, qT_ca, kvT_ca, kvcolT_ca,
                           cb["ca_wf"], c_negblk, ident_bf, c_lneps,
                           attn_ca)
            with ExitStack() as ph:
                _proj_add(nc, tc, ph, attn_ca, w_o2, cb["ca_bo"], x_t)
            cabk.close()

            # ---------- LN3 + FFN (4 quarters of F) + residual in x_t
            w1 = _bslice(blob, "w1").rearrange("p (q k d) -> p q k d", q=4,
                                               k=KD)
            w2 = _bslice(blob, "w2").rearrange("p (q k d) -> p q k d", q=4,
                                               k=KD)
            with ExitStack() as ph:
                sb = ph.enter_context(tc.tile_pool(name="ffsb", bufs=3))
                wfp = ph.enter_context(tc.tile_pool(name="ffwp", bufs=2))
                h1p = ph.enter_context(tc.tile_pool(name="h1p", bufs=1))
                # prefetch quarter-0 weights so the DMAs ride under LN3
                w1q0 = wfp.tile([128, KD, 1024], bf16, tag="wffn",
                                name="w1q0")
                nc.sync.dma_start(out=w1q0[:], in_=w1[:, 0, :, :])
                w2q0 = wfp.tile([128, KD, 1024], bf16, tag="wffn",
                                name="w2q0")
                nc.sync.dma_start(out=w2q0[:], in_=w2[:, 0, :, :])
                with ExitStack() as lnph:
                    _ln(nc, tc, lnph, x_t, t2_t, c_invd_r, c_eps)
                ps1 = ph.enter_context(tc.tile_pool(name="f1ps", bufs=2,
                                                    space="PSUM"))
                ps2 = ph.enter_context(tc.tile_pool(name="f2ps", bufs=2,
                                                    space="PSUM"))
                for q in range(4):
                    if q == 0:
                        w1q, w2q = w1q0, w2q0
                    else:
                        w1q = wfp.tile([128, KD, 1024], bf16, tag="wffn",
                                       name="w1q")
                        nc.sync.dma_start(out=w1q[:], in_=w1[:, q, :, :])
                        w2q = wfp.tile([128, KD, 1024], bf16, tag="wffn",
                                       name="w2q")
                        nc.sync.dma_start(out=w2q[:], in_=w2[:, q, :, :])
                    h1 = h1p.tile([128, KD, SL], bf16, tag="h1", name="h1")
                    for m in range(KD):
                        o_ps = ps1.tile([128, SL], f32, tag="f1", name="f1")
                        for kd in range(KD):
                            for ch in range(NCH):
                                cs = bass.ts(ch, 512)
                                nc.tensor.matmul(
                                    o_ps[:, cs],
                                    w1q[:, kd, m * 128:(m + 1) * 128],
                                    t2_t[:, kd, cs],
                                    start=(kd == 0), stop=(kd == KD - 1),
                                    skip_group_check=True)
                        nc.scalar.activation(
                            h1[:, m, :], o_ps[:], AF.Relu,
                            bias=cb["b1"][:, q * 8 + m:q * 8 + m + 1])
                    for m in range(KD):
                        o_ps = ps2.tile([128, SL], f32, tag="f2", name="f2")
                        for kf in range(KD):
                            for ch in range(NCH):
                                cs = bass.ts(ch, 512)
                                nc.tensor.matmul(
                                    o_ps[:, cs],
                                    w2q[:, kf, m * 128:(m + 1) * 128],
                                    h1[:, kf, cs],
                                    start=(kf == 0), stop=(kf == KD - 1),
                                    skip_group_check=True)
                        if q == 0:
                            nc.vector.scalar_tensor_tensor(
                                x_t[:, m, :], o_ps[:],
                                cb["b2"][:, m:m + 1], x_t[:, m, :],
                                ALU.add, ALU.add)
                        else:
                            nc.any.tensor_add(x_t[:, m, :],
                                              x_t[:, m, :], o_ps[:])
                            if q == 3:
                                # stream the finished m-tile out
                                nc.sync.dma_start(
                                    out=outT[:, m, :],
                                    in_=x_t[:, m, :].bitcast(f32))
    nc.finalize()
    return nc


# ------------------------------------------------------------------ host

def _prep_inputs(inputs):
    Cs = DH ** -0.25
    f = np.float32
    bf = ml_dtypes.bfloat16
    inp = {k: np.asarray(v, dtype=f) for k, v in inputs.items()}

    def fshape(vec):
        n = vec.shape[0] // 128
        return np.ascontiguousarray(vec.reshape(n, 128).T)

    def wpack(w_t):
        # (din, dout) -> (128, KD, dout)
        dout = w_t.shape[1]
        return np.ascontiguousarray(
            w_t.reshape(KD, 128, dout).transpose(1, 0, 2))

    def cols(arr):
        """(128, ...) array of f32/bf16 -> (128, n) bf16 byte view."""
        a = np.ascontiguousarray(arr)
        a = a.reshape(128, -1)
        return a.view(bf)

    shared = {}
    shared["c_ident"] = np.eye(128, dtype=bf)

    # fold LN gamma/beta into the consumers of each LN output
    ln_fold = {"sa_wq": "1", "sa_wk": "1", "sa_wv": "1", "ca_wq": "2"}
    bias_of = {"wq": "bq", "wk": "bk", "wv": "bv"}
    biases = {p + "_" + b: inp[p + "_" + b].copy()
              for p in ("sa", "ca") for b in ("bq", "bk", "bv", "bo")}
    for pre in ("sa", "ca"):
        for nm in ("wq", "wk", "wv", "wo"):
            key = pre + "_" + nm
            w_t = np.ascontiguousarray(inp[key].T)
            if key in ln_fold:
                i = ln_fold[key]
                w_t = w_t * inp["ln%s_g" % i][:, None]
                biases[pre + "_" + bias_of[nm]] += (
                    inp[key] @ inp["ln%s_b" % i])
            shared[key] = wpack(w_t).astype(bf)
        for nm in ("bq", "bk", "bo"):
            shared[pre + "_" + nm] = fshape(biases[pre + "_" + nm])
        bvrow = np.zeros((128, D), bf)
        bvrow[0, :] = biases[pre + "_bv"].astype(bf)
        shared[pre + "_bv"] = bvrow
        wf_t = (Cs * inp[pre + "_feat"]).T          # (DH, M)
        wfc = np.zeros((128, 512), f)
        wfc[0:64, 0:256] = wf_t
        wfc[64:128, 256:512] = wf_t
        shared[pre + "_wf"] = wfc.astype(bf)
    a = np.ascontiguousarray(inp["ff_w1"].T) * inp["ln3_g"][:, None]
    b1_fold = inp["ff_b1"] + inp["ff_w1"] @ inp["ln3_b"]
    shared["w1"] = np.ascontiguousarray(
        a.reshape(KD, 128, 4, 1024).transpose(1, 2, 0, 3)).astype(bf)
    b = np.ascontiguousarray(inp["ff_w2"].T)            # (F, D)
    shared["w2"] = np.ascontiguousarray(
        b.reshape(4, KD, 128, 1024).transpose(2, 0, 1, 3)).astype(bf)
    shared["b1"] = fshape(b1_fold)
    shared["b2"] = fshape(inp["ff_b2"])

    shared_cols = {nm: cols(shared[nm]) for nm in shared}

    in_maps = []
    for core in range(8):
        b_ix, half = core // 2, core % 2
        sl = slice(half * SL, (half + 1) * SL)
        xt = np.ascontiguousarray(inp["tgt"][sl, b_ix, :].T)      # (D, SL)
        xT = np.ascontiguousarray(
            xt.reshape(KD, 128, SL).transpose(1, 0, 2))
        mt = np.ascontiguousarray(inp["memory"][sl, b_ix, :].T)
        memT = np.ascontiguousarray(
            mt.reshape(KD, 128, SL).transpose(1, 0, 2)).astype(bf)
        percore = {"xT": cols(xT), "memT": cols(memT)}
        blocks = []
        for nm, ncols in _BLOB_SPEC:
            blk = percore.get(nm)
            if blk is None:
                blk = shared_cols[nm]
            assert blk.shape == (128, ncols), (nm, blk.shape, ncols)
            blocks.append(blk)
        in_maps.append({"blob": np.concatenate(blocks, axis=1)})
    return in_maps


def _build_exec(nc, n_cores=8):
    import jax
    from jax.sharding import Mesh, PartitionSpec
    from jax.experimental.shard_map import shard_map
    from concourse import bass2jax as b2j

    b2j.install_neuronx_cc_hook()
    partition_name = (nc.partition_id_tensor.name
                      if nc.partition_id_tensor else None)
    in_names, out_names, out_avals = [], [], []
    for alloc in nc.m.functions[0].allocations:
        if not isinstance(alloc, mybir.MemoryLocationSet):
            continue
        name = alloc.memorylocations[0].name
        if alloc.kind == "ExternalInput":
            if name != partition_name:
                in_names.append(name)
        elif alloc.kind == "ExternalOutput":
            out_names.append(name)
            out_avals.append(jax.core.ShapedArray(
                tuple(alloc.tensor_shape), mybir.dt.np(alloc.dtype)))
    n_params = len(in_names)
    all_in = list(in_names) + list(out_names)
    if partition_name is not None:
        all_in.append(partition_name)

    def _body(*args):
        operands = list(args)
        if partition_name is not None:
            operands.append(b2j.partition_id_tensor())
        outs = b2j._bass_exec_p.bind(
            *operands, out_avals=tuple(out_avals), in_names=tuple(all_in),
            out_names=tuple(out_names), lowering_input_output_aliases=(),
            sim_require_finite=True, sim_require_nnan=True, nc=nc)
        return tuple(outs)

    devices = jax.devices()[:n_cores]
    mesh = Mesh(np.asarray(devices), ("core",))
    n_outs = len(out_names)
    specs = (PartitionSpec("core"),) * (n_params + n_outs)
    out_specs = (PartitionSpec("core"),) * n_outs
    donate = tuple(range(n_params, n_params + n_outs))
    sharded = jax.jit(shard_map(_body, mesh=mesh, in_specs=specs,
                                out_specs=out_specs, check_rep=False),
                      donate_argnums=donate, keep_unused=True)

    def run(in_maps, fetch=True):
        import jax as _jax
        concat = [np.concatenate([np.asarray(in_maps[c][nm])
                                  for c in range(n_cores)], axis=0)
                  for nm in in_names]
        zeros = [np.zeros((n_cores * av.shape[0], *av.shape[1:]), av.dtype)
                 for av in out_avals]
        outs = sharded(*concat, *zeros)
        if not fetch:
            _jax.block_until_ready(outs)
            return None
        return [{nm: np.asarray(outs[i]).reshape(
            n_cores, *out_avals[i].shape)[c]
            for i, nm in enumerate(out_names)} for c in range(n_cores)]

    def time_exec(in_maps, iters=8):
        """Wall-time the sharded exec with device-resident inputs."""
        import time as _time
        import jax as _jax
        from jax.sharding import NamedSharding
        sh = NamedSharding(mesh, PartitionSpec("core"))
        concat = [np.concatenate([np.asarray(in_maps[c][nm])
                                  for c in range(n_cores)], axis=0)
                  for nm in in_names]
        dev_in = _jax.device_put(concat, [sh] * len(concat))
        _jax.block_until_ready(dev_in)
        zeros = [np.zeros((n_cores * av.shape[0], *av.shape[1:]), av.dtype)
                 for av in out_avals]
        times = []
        for _ in range(iters):
            zd = _jax.device_put(zeros, [sh] * len(zeros))
            _jax.block_until_ready(zd)
            t0 = _time.time()
            outs = sharded(*dev_in, *zd)
            _jax.block_until_ready(outs)
            times.append(_time.time() - t0)
        return times

    run.in_names = in_names
    run.time_exec = time_exec
    run.sharded = sharded
    run.mesh = mesh
    run.out_avals = out_avals
    run.n_params = n_params
    return run


def _get_exec():
    if "exec" not in _CACHE:
        nc = build_nc()
        _CACHE["exec"] = _build_exec(nc)
    return _CACHE["exec"]


def kernel(**inputs):
    run = _get_exec()
    in_maps = _prep_inputs(inputs)
    res = run(in_maps)
    out = np.empty((S, B, D), np.float32)
    for c in range(8):
        b_ix, half = c // 2, c % 2
        slab = res[c]["outT"]                       # (128, KD, SL)
        out[half * SL:(half + 1) * SL, b_ix, :] = (
            slab.transpose(1, 0, 2).reshape(D, SL).T)
    return out
